# revision 4
# baseline (speedup 1.0000x reference)
"""GAT layer (nn_GAT_40037685133531) as a Trainium2 Bass kernel on 8 NeuronCores.

v4 strategy (graph/data parallel, no collectives):
  - Destination nodes sharded 8 ways (6250 per core); edges grouped by the
    128-node destination tile owning their dst, packed into K slots of 128.
  - Phase 0 (replicated, bf16): h_ext = x @ [W | W@A_s | W@A_d] -> htab
    [NPAD, 136] bf16 rows = [h bf16 x128 | alpha_s f32 x4] (272B); alpha_d
    accumulated in SBUF and written once to ad_tab [NPAD, 4] f32.
  - Phase 1 per tile: K x [P,1]-offset indirect DMAs gather the edge rows
    by src (measured: ~1.1us/op fixed issue cost on gpsimd is the kernel
    floor; multi-offset/dma_gather/ap_gather forms all measured slower).
    alpha_d for the tile's 128 dst nodes comes from one [P,1] gather on
    ad_tab and is broadcast to edges via bf16 PE transposes of the one-hot
    S (built by DVE is_equal vs an iota). ex = exp(leakyrelu(as+ad)),
    msgs = [ex*h | ex] bf16, psum += S_j.T @ msgs_j aggregates weighted
    sums + softmax denominators. Normalize, ELU, PE-transpose, z @ W2 in
    bf16; y accumulated in SBUF and written once.
"""

import os
import sys

import numpy as np

if "/opt/trn_rl_repo" not in sys.path:
    sys.path.insert(0, "/opt/trn_rl_repo")

N_NODES = 50000
N_EDGES = 800000
F_IN = 128
HEADS = 4
HIDDEN = 32
F_OUT = 64
NEG = 0.2
N_CORES = 8
P = 128
FE = F_IN + 2 * HEADS    # 136 phase-0 psum cols: h | alpha_s | alpha_d
FM = F_IN + HEADS        # 132 message cols: ex*h | ex
EB = FE                  # bf16 elements per htab row (272 B)
AS_OFF = 64              # f32 element offset of alpha_s within a row
NODES_PER_CORE = N_NODES // N_CORES          # 6250
T_TILES = (NODES_PER_CORE + P - 1) // P      # 49
NPAD = T_TILES * 8 * P                       # 50176
CH = 14                                      # phase-0 node tiles per chunk

def _prep(edge_index, W, a_src, a_dst):
    """CPU-side: extended weights; edges sorted by (core, tile, src) and
    packed into [P, K] slot layout per destination tile."""
    src = np.asarray(edge_index[0]).astype(np.int64)
    dst = np.asarray(edge_index[1]).astype(np.int64)

    A_s = np.zeros((F_IN, HEADS), np.float32)
    A_d = np.zeros((F_IN, HEADS), np.float32)
    for h in range(HEADS):
        A_s[h * HIDDEN:(h + 1) * HIDDEN, h] = a_src[h]
        A_d[h * HIDDEN:(h + 1) * HIDDEN, h] = a_dst[h]
    W_ext = np.concatenate([W, W @ A_s, W @ A_d], axis=1).astype(np.float32)

    core_of = dst // NODES_PER_CORE
    # Bin-pack each core's dst nodes into T_TILES groups of exactly P nodes,
    # balancing edge counts (LPT): per-core edges avg NODES... ~100k/49 ~= 2041
    # < 2048, so nearly every group fits 16 slots of 128 edges.
    node_group = np.zeros((N_CORES, NODES_PER_CORE), np.int32)
    node_pos = np.zeros((N_CORES, NODES_PER_CORE), np.int32)
    group_nodes = np.zeros((N_CORES, T_TILES, P), np.int64)
    pad_from = np.zeros((N_CORES, T_TILES), np.int32)
    for c in range(N_CORES):
        lo, hi = c * NODES_PER_CORE, (c + 1) * NODES_PER_CORE
        deg = np.bincount(dst[core_of == c] - lo, minlength=NODES_PER_CORE)
        order_n = np.argsort(-deg, kind="stable")
        # serpentine deal for near-equal loads with equal node counts
        bins = [[] for _ in range(T_TILES)]
        bi, step = 0, 1
        for ln in order_n:
            while len(bins[bi]) >= P:
                bi += step
                if bi in (-1, T_TILES):
                    step = -step
                    bi += step
            bins[bi].append(ln)
            bi += step
            if bi in (-1, T_TILES):
                step = -step
                bi += step
        load = np.array([int(deg[b].sum()) for b in bins])
        # repair: push overflow (>16 slots) into the single heaviest bin by
        # swapping its light nodes for other bins' heavy nodes
        CAP = 16 * P
        b0 = int(np.argmax(load))
        for b in range(T_TILES):
            if b == b0:
                continue
            while load[b] > CAP:
                hi_i = max(range(len(bins[b])), key=lambda i: deg[bins[b][i]])
                lo_i = min(range(len(bins[b0])),
                           key=lambda i: deg[bins[b0][i]])
                nh, nl = bins[b][hi_i], bins[b0][lo_i]
                if deg[nh] <= deg[nl]:
                    break
                bins[b][hi_i], bins[b0][lo_i] = nl, nh
                load[b] += deg[nl] - deg[nh]
                load[b0] += deg[nh] - deg[nl]
        nfill = np.zeros(T_TILES, np.int32)
        for b in range(T_TILES):
            for pos, ln in enumerate(bins[b]):
                node_group[c, ln] = b
                node_pos[c, ln] = pos
                group_nodes[c, b, pos] = lo + ln
            nfill[b] = len(bins[b])
        # pad slots (tiles of the last 22 dummies) already absorbed: every
        # bin has exactly P real nodes (NODES_PER_CORE=6250 < 49*128=6272)
        for b in np.flatnonzero(nfill < P):
            group_nodes[c, b, nfill[b]:] = lo  # harmless duplicate for adt
        pad_from[c] = nfill

    group_all = (core_of * T_TILES
                 + node_group[core_of, dst - core_of * NODES_PER_CORE])
    order = np.lexsort((src, group_all))
    src_s, dst_s, group_s = src[order], dst[order], group_all[order]

    NG = N_CORES * T_TILES
    gs = np.searchsorted(group_s, np.arange(NG))
    ge = np.searchsorted(group_s, np.arange(NG), side="right")
    cnt = (ge - gs).reshape(N_CORES, T_TILES)
    # Sort each core's groups by edge count (desc): loop position t then only
    # needs k_list[t] = max over cores of the t-th largest slot count.
    perm = np.argsort(-cnt, axis=1)
    cnt_sorted = np.take_along_axis(cnt, perm, axis=1)
    k_list = [max(1, int(np.max((cnt_sorted[:, t] + P - 1) // P)))
              for t in range(T_TILES)]
    K = max(k_list)

    src32 = np.zeros((N_CORES, T_TILES, P, K), np.int32)
    d_local = np.full((N_CORES, T_TILES, P, K), -1.0, np.float32)
    dst_nodes = np.zeros((N_CORES, T_TILES, P, 1), np.int32)
    node_order = np.zeros((N_CORES, T_TILES, P), np.int64)
    # dlocT[c, t, j*P + e] = d_local[c, t, e, j] (edge-slot-major, for STt)
    for c in range(N_CORES):
        for tp in range(T_TILES):
            t = perm[c, tp]
            g = c * T_TILES + t
            s, e = gs[g], ge[g]
            n = e - s
            i = np.arange(n)
            dl = node_pos[c, dst_s[s:e] - c * NODES_PER_CORE]
            src32[c, tp, i % P, i // P] = src_s[s:e]
            d_local[c, tp, i % P, i // P] = dl
            dst_nodes[c, tp, :, 0] = group_nodes[c, t]
            node_order[c, tp] = group_nodes[c, t]
            if pad_from[c, t] < P:
                node_order[c, tp, pad_from[c, t]:] = -1
    dlocT = np.ascontiguousarray(
        d_local.transpose(0, 1, 3, 2)).reshape(N_CORES, T_TILES, K * P)
    return W_ext, src32, d_local, dlocT, dst_nodes, node_order, k_list


def _build_module(k_list, bias_nz, b2_nz):
    K = max(k_list)
    import concourse.bass as bass
    import concourse.mybir as mybir
    import concourse.tile as tile
    from concourse import bacc
    from concourse.masks import make_identity

    f32 = mybir.dt.float32
    bf16 = mybir.dt.bfloat16
    i32 = mybir.dt.int32

    nc = bacc.Bacc("TRN2", target_bir_lowering=False, debug=False,
                   num_devices=N_CORES, dynamic_dma_scratch_size=32768)

    x_T = nc.dram_tensor("x_T", [P, NPAD], bf16, kind="ExternalInput")
    W_ext_d = nc.dram_tensor("W_ext", [P, FE], bf16, kind="ExternalInput")
    W2_d = nc.dram_tensor("W2", [P, F_OUT], bf16, kind="ExternalInput")
    s32_d = nc.dram_tensor("src32", [T_TILES, P, K], i32,
                           kind="ExternalInput")
    dstn_d = nc.dram_tensor("dst_nodes", [T_TILES, P, 1], i32,
                            kind="ExternalInput")
    dloc_d = nc.dram_tensor("d_local", [T_TILES, P, K], bf16,
                            kind="ExternalInput")
    dlocT_d = nc.dram_tensor("d_localT", [T_TILES, K * P], bf16,
                             kind="ExternalInput")
    if bias_nz:
        bias_d = nc.dram_tensor("bias_ext", [1, FE], bf16,
                                kind="ExternalInput")
    if b2_nz:
        b2_d = nc.dram_tensor("b2_row", [1, F_OUT], bf16,
                              kind="ExternalInput")
    y_d = nc.dram_tensor("y_out", [T_TILES * P, F_OUT], f32,
                         kind="ExternalOutput")
    htab = nc.dram_tensor("htab", [NPAD, EB], bf16, kind="Internal")
    ad_tab = nc.dram_tensor("ad_tab", [NPAD, HEADS], f32, kind="Internal")

    add = mybir.AluOpType.add
    mult = mybir.AluOpType.mult
    amax = mybir.AluOpType.max
    is_eq = mybir.AluOpType.is_equal
    Exp = mybir.ActivationFunctionType.Exp

    N_CHUNKS = NPAD // (CH * P)  # 28

    with tile.TileContext(nc) as tc:
        with tc.tile_pool(name="const", bufs=1) as constp:
            W_ext_sb = constp.tile([P, FE], bf16)
            nc.sync.dma_start(W_ext_sb[:], W_ext_d.ap())
            W2_sb = constp.tile([P, F_OUT], bf16)
            nc.sync.dma_start(W2_sb[:], W2_d.ap())
            iota_f = constp.tile([P, P], f32)
            nc.gpsimd.iota(iota_f[:], pattern=[[1, P]], base=0,
                           channel_multiplier=0,
                           allow_small_or_imprecise_dtypes=True)
            iota_sb = constp.tile([P, P], bf16)
            nc.vector.tensor_copy(iota_sb[:], iota_f[:])
            iotap_f = constp.tile([P, 1], f32)
            nc.gpsimd.iota(iotap_f[:], pattern=[[0, 1]], base=0,
                           channel_multiplier=1,
                           allow_small_or_imprecise_dtypes=True)
            iotap = constp.tile([P, 1], bf16)
            nc.vector.tensor_copy(iotap[:], iotap_f[:])
            ident_f = constp.tile([P, P], f32)
            make_identity(nc, ident_f[:])
            ident = constp.tile([P, P], bf16)
            nc.vector.tensor_copy(ident[:], ident_f[:])
            ones1 = constp.tile([1, P], bf16)
            nc.vector.memset(ones1[:], 1.0)
            s32_sb = constp.tile([P, T_TILES, K], i32)
            nc.sync.dma_start(s32_sb[:],
                              s32_d.ap().rearrange("t p k -> p t k"))
            dstn_sb = constp.tile([P, T_TILES], i32)
            nc.sync.dma_start(dstn_sb[:],
                              dstn_d.ap().rearrange("t p one -> p (t one)"))
            dloc_sb = constp.tile([P, T_TILES, K], bf16)
            nc.sync.dma_start(dloc_sb[:],
                              dloc_d.ap().rearrange("t p k -> p t k"))
            if bias_nz or b2_nz:
                ones_sb = constp.tile([1, P], bf16)
                nc.vector.memset(ones_sb[:], 1.0)
            if bias_nz:
                bias_sb = constp.tile([1, FE], bf16)
                nc.sync.dma_start(bias_sb[:], bias_d.ap())
            if b2_nz:
                b2_sb = constp.tile([1, F_OUT], bf16)
                nc.sync.dma_start(b2_sb[:], b2_d.ap())
            ad_acc = constp.tile([P, NPAD // P, HEADS], f32)
            y_acc = constp.tile([P, T_TILES, F_OUT], f32)

            # ---- phase 0: htab = [x@W_ext | as]; ad_acc = ad ----
            # 3 node-tiles share one PSUM bank so the PSUM->SBUF copies
            # amortize the DVE per-op overhead.
            with (
                tc.tile_pool(name="xt", bufs=3) as xtp,
                tc.tile_pool(name="hx", bufs=3) as hxp,
                tc.tile_pool(name="p0ps", bufs=4, space="PSUM") as p0ps,
            ):
                for c in range(N_CHUNKS):
                    xt = xtp.tile([P, CH * P], bf16)
                    nc.scalar.dma_start(
                        xt[:], x_T.ap()[:, c * CH * P:(c + 1) * CH * P])
                    hrow = hxp.tile([P, CH, EB], bf16, tag="hrow")
                    hrow_f32 = hrow[:].bitcast(f32)
                    for j0 in range(0, CH, 3):
                        nj = min(3, CH - j0)
                        ps = p0ps.tile([P, 3, FE], f32)
                        for j in range(j0, j0 + nj):
                            nc.tensor.matmul(
                                ps[:, j - j0, :],
                                lhsT=xt[:, j * P:(j + 1) * P],
                                rhs=W_ext_sb[:], start=True,
                                stop=not bias_nz)
                            if bias_nz:
                                nc.tensor.matmul(ps[:, j - j0, :],
                                                 lhsT=ones_sb[:],
                                                 rhs=bias_sb[:], start=False,
                                                 stop=True)
                        nc.vector.tensor_copy(
                            hrow[:, j0:j0 + nj, 0:F_IN],
                            ps[:, 0:nj, 0:F_IN])
                        nc.vector.tensor_copy(
                            hrow_f32[:, j0:j0 + nj, AS_OFF:AS_OFF + HEADS],
                            ps[:, 0:nj, F_IN:F_IN + HEADS])
                        nc.vector.tensor_copy(
                            ad_acc[:, c * CH + j0:c * CH + j0 + nj, :],
                            ps[:, 0:nj, F_IN + HEADS:FE])
                    rows = slice(c * CH * P, (c + 1) * CH * P)
                    nc.sync.dma_start(
                        htab.ap()[rows, :].rearrange("(t p) e -> p t e", p=P),
                        hrow[:])
                    if (c + 1) % 7 == 0:  # quarters: after chunks 6,13,20,27
                        q = slice((c - 6) * CH * P, (c + 1) * CH * P)
                        nc.sync.dma_start(
                            ad_tab.ap()[q, :].rearrange(
                                "(t p) e -> p t e", p=P),
                            ad_acc[:, (c - 6) * CH:(c + 1) * CH, :])

            # ---- phase 1: per destination tile ----
            BC = 512  # bcast-matmul chunk (one PSUM bank of f32)
            with (
                tc.tile_pool(name="g", bufs=6) as gp,
                tc.tile_pool(name="msgs", bufs=3) as mp,
                tc.tile_pool(name="S", bufs=2) as sp,
                tc.tile_pool(name="STt", bufs=2) as stp,
                tc.tile_pool(name="dlT", bufs=2) as dlp,
                tc.tile_pool(name="agg", bufs=2, space="PSUM") as aggp,
                tc.tile_pool(name="bcps", bufs=2, space="PSUM") as bcpsp,
                tc.tile_pool(name="adps", bufs=2, space="PSUM") as adpsp,
                tc.tile_pool(name="small", bufs=4) as smallp,
                tc.tile_pool(name="tr", bufs=1, space="PSUM") as trp,
                tc.tile_pool(name="yps", bufs=1, space="PSUM") as ypsp,
            ):
                for t in range(T_TILES):
                    Kt = k_list[t]
                    adt = smallp.tile([P, HEADS], f32, tag="adt")
                    nc.gpsimd.indirect_dma_start(
                        out=adt[:], out_offset=None, in_=ad_tab.ap(),
                        in_offset=bass.IndirectOffsetOnAxis(
                            ap=dstn_sb[:, t:t + 1], axis=0))
                    g = gp.tile([P, Kt, EB], bf16, tag="g")
                    for j in range(Kt):
                        nc.gpsimd.indirect_dma_start(
                            out=g[:, j, :], out_offset=None,
                            in_=htab.ap(),
                            in_offset=bass.IndirectOffsetOnAxis(
                                ap=s32_sb[:, t, j:j + 1], axis=0))
                    adtb = smallp.tile([P, HEADS], bf16, tag="adtb")
                    nc.vector.tensor_copy(adtb[:], adt[:])

                    # one-hot scatter matrix S[p=e, (k, d)] in bf16
                    S = sp.tile([P, Kt, P], bf16, tag="S")
                    nc.vector.tensor_tensor(
                        out=S[:],
                        in0=iota_sb[:].unsqueeze(1).to_broadcast([P, Kt, P]),
                        in1=dloc_sb[:, t, 0:Kt].unsqueeze(2).to_broadcast(
                            [P, Kt, P]),
                        op=is_eq)
                    # STt[d, (j e)] = (d == dloc[e, j]) built directly:
                    # dlocT row broadcast across partitions via PE, then one
                    # is_equal against the partition-index iota.
                    dlT_row = dlp.tile([1, Kt * P], bf16, tag="dlr")
                    nc.sync.dma_start(dlT_row[:],
                                      dlocT_d.ap()[t:t + 1, 0:Kt * P])
                    dlT = dlp.tile([P, Kt * P], bf16, tag="dlT")
                    for q0 in range(0, Kt * P, BC):
                        qn = min(BC, Kt * P - q0)
                        bps = bcpsp.tile([P, BC], f32)
                        nc.tensor.matmul(bps[:, 0:qn], lhsT=ones1[:],
                                         rhs=dlT_row[:, q0:q0 + qn],
                                         start=True, stop=True)
                        nc.vector.tensor_copy(dlT[:, q0:q0 + qn],
                                              bps[:, 0:qn])
                    STt = stp.tile([P, Kt * P], bf16, tag="STt")
                    nc.vector.tensor_tensor(
                        out=STt[:],
                        in0=iotap[:].to_broadcast([P, Kt * P]),
                        in1=dlT[:], op=is_eq)
                    # alpha_d per edge: adps[:, j*4:(j+1)*4] = STt_j.T @ adtb
                    adps = adpsp.tile([P, Kt * HEADS], f32, tag="adps")
                    for j in range(Kt):
                        nc.tensor.matmul(
                            adps[:, j * HEADS:(j + 1) * HEADS],
                            lhsT=STt[:, j * P:(j + 1) * P], rhs=adtb[:],
                            start=True, stop=True)

                    # ex = exp(leakyrelu(alpha_s + alpha_d)) per edge
                    gf = g[:].bitcast(f32)   # [P, Kt, 68]
                    ex = smallp.tile([P, Kt, HEADS], f32, tag="ex")
                    nc.vector.tensor_tensor(
                        out=ex[:], in0=gf[:, :, AS_OFF:AS_OFF + HEADS],
                        in1=adps[:].rearrange("p (k h) -> p k h", k=Kt),
                        op=add)
                    nc.vector.scalar_tensor_tensor(
                        out=ex[:], in0=ex[:], scalar=NEG, in1=ex[:],
                        op0=mult, op1=amax)
                    exb = smallp.tile([P, Kt, HEADS], bf16, tag="exb")
                    nc.scalar.activation(out=exb[:], in_=ex[:], func=Exp)

                    msgs = mp.tile([P, Kt, FM], bf16, tag="msgs")
                    nc.vector.tensor_tensor(
                        out=msgs[:, :, 0:F_IN].rearrange(
                            "p k (h f) -> p k h f", h=HEADS),
                        in0=g[:, :, 0:F_IN].rearrange(
                            "p k (h f) -> p k h f", h=HEADS),
                        in1=exb[:].unsqueeze(3).to_broadcast(
                            [P, Kt, HEADS, HIDDEN]),
                        op=mult)
                    nc.vector.tensor_copy(msgs[:, :, F_IN:FM], exb[:])

                    ps = aggp.tile([P, FM], f32)
                    for j in range(Kt):
                        nc.tensor.matmul(ps[:], lhsT=S[:, j, :],
                                         rhs=msgs[:, j, :],
                                         start=(j == 0), stop=(j == K - 1))

                    rec = smallp.tile([P, HEADS], f32, tag="rec")
                    nc.vector.tensor_scalar_add(out=rec[:],
                                                in0=ps[:, F_IN:FM],
                                                scalar1=1e-16)
                    nc.vector.reciprocal(rec[:], rec[:])
                    zn = smallp.tile([P, F_IN], f32, tag="zn")
                    nc.vector.tensor_tensor(
                        out=zn[:].rearrange("p (h f) -> p h f", h=HEADS),
                        in0=ps[:, 0:F_IN].rearrange("p (h f) -> p h f",
                                                    h=HEADS),
                        in1=rec[:].unsqueeze(2).to_broadcast(
                            [P, HEADS, HIDDEN]),
                        op=mult)
                    # ELU(z) = max(z, exp(min(z,0)) - 1)
                    tmp = smallp.tile([P, F_IN], f32, tag="tmp")
                    nc.vector.tensor_scalar_min(out=tmp[:], in0=zn[:],
                                                scalar1=0.0)
                    nc.scalar.activation(out=tmp[:], in_=tmp[:], func=Exp)
                    znb = smallp.tile([P, F_IN], bf16, tag="znb")
                    nc.vector.scalar_tensor_tensor(
                        out=znb[:], in0=tmp[:], scalar=-1.0, in1=zn[:],
                        op0=add, op1=amax)

                    pt = trp.tile([P, P], bf16, tag="pt")
                    nc.tensor.transpose(out=pt[:], in_=znb[:],
                                        identity=ident[:])
                    znT = smallp.tile([P, P], bf16, tag="znT")
                    nc.vector.tensor_copy(znT[:], pt[:])
                    yp = ypsp.tile([P, F_OUT], f32, tag="yp")
                    nc.tensor.matmul(yp[:], lhsT=znT[:], rhs=W2_sb[:],
                                     start=True, stop=not b2_nz)
                    if b2_nz:
                        nc.tensor.matmul(yp[:], lhsT=ones_sb[:], rhs=b2_sb[:],
                                         start=False, stop=True)
                    nc.vector.tensor_copy(y_acc[:, t, :], yp[:])
            nc.sync.dma_start(
                y_d.ap().rearrange("(t p) f -> p t f", p=P), y_acc[:])

    nc.compile()
    return nc


_MODULE_CACHE = {}


def _get_module(k_list, bias_nz, b2_nz):
    key = (tuple(k_list), bias_nz, b2_nz)
    if key not in _MODULE_CACHE:
        _MODULE_CACHE[key] = _build_module(k_list, bias_nz, b2_nz)
    return _MODULE_CACHE[key]


def _ensure_ntff_hook():
    """The axon NTFF profile hook lives in antenv.axon_hooks, which this
    image's antenv package lacks; shim it so trace=True works."""
    try:
        import antenv.axon_hooks  # noqa: F401
        return
    except ImportError:
        pass
    import types

    import antenv

    mod = types.ModuleType("antenv.axon_hooks")
    holder = {"h": None}
    mod.set_axon_ntff_profile_hook = lambda h: holder.__setitem__("h", h)
    mod.get_axon_ntff_profile_hook = lambda: holder["h"]
    try:
        from trn_agent_boot.trn_boot import _ntff_profile_via_ctypes
        holder["h"] = _ntff_profile_via_ctypes("/opt/axon/libaxon_pjrt.so")
    except Exception:
        pass
    sys.modules["antenv.axon_hooks"] = mod
    antenv.axon_hooks = mod


def kernel(x, edge_index, edge_weight, W, a_src, a_dst, bias, W2, b2,
           _trace=False):
    import ml_dtypes
    from concourse.bass_utils import run_bass_kernel_spmd

    bf = ml_dtypes.bfloat16
    if _trace:
        _ensure_ntff_hook()

    x = np.asarray(x, np.float32)
    W = np.asarray(W, np.float32)
    a_src = np.asarray(a_src, np.float32)
    a_dst = np.asarray(a_dst, np.float32)
    bias = np.asarray(bias, np.float32)
    W2 = np.asarray(W2, np.float32)
    b2 = np.asarray(b2, np.float32)

    W_ext, src32, d_local, dlocT, dst_nodes, node_order, k_list = _prep(
        edge_index, W, a_src, a_dst)

    bias_nz = bool(np.any(bias))
    b2_nz = bool(np.any(b2))
    nc = _get_module(k_list, bias_nz, b2_nz)

    x_T = np.zeros((P, NPAD), bf)
    x_T[:, :N_NODES] = x.T.astype(bf)

    in_maps = []
    for c in range(N_CORES):
        m = {
            "x_T": x_T,
            "W_ext": W_ext.astype(bf),
            "W2": W2.astype(bf),
            "src32": np.ascontiguousarray(src32[c]),
            "dst_nodes": np.ascontiguousarray(dst_nodes[c]),
            "d_local": np.ascontiguousarray(d_local[c].astype(bf)),
            "d_localT": np.ascontiguousarray(dlocT[c].astype(bf)),
        }
        if bias_nz:
            be = np.zeros((1, FE), np.float32)
            be[0, :F_IN] = bias
            m["bias_ext"] = be.astype(bf)
        if b2_nz:
            m["b2_row"] = b2.reshape(1, F_OUT).astype(bf)
        in_maps.append(m)

    res = run_bass_kernel_spmd(nc, in_maps, core_ids=list(range(N_CORES)),
                               trace=_trace)
    out = np.zeros((N_NODES, F_OUT), np.float32)
    for c in range(N_CORES):
        y = res.results[c]["y_out"].reshape(T_TILES * P, F_OUT)
        idx = node_order[c].reshape(-1)
        valid = idx >= 0
        out[idx[valid]] = y[valid]
    if _trace:
        kernel.last_results = res
    return out


# revision 5
# speedup vs baseline: 1.1139x; 1.1139x over previous
"""GAT layer (nn_GAT_40037685133531) as a Trainium2 Bass kernel on 8 NeuronCores.

Strategy (graph/data parallel, no collectives):
  - Destination nodes sharded 8 ways (6250 per core), then bin-packed into
    49 balanced 128-node groups per core (serpentine deal + overflow
    concentrated into one group) so 48 of 49 groups need exactly 16
    edge slots of 128; per-position slot counts are the max over cores.
  - Phase 0 (replicated, bf16): h_ext = x @ [W | W@A_s | W@A_d] -> htab
    [NPAD, 136] bf16 rows = [h bf16 x128 | alpha_s f32 x4] (272B); alpha_d
    accumulated in SBUF and written once to ad_tab [NPAD, 4] f32.
  - Phase 1 per tile: K x [P,1]-offset indirect DMAs gather the edge rows
    by src (measured: ~1.1us/op fixed issue cost on gpsimd is the kernel
    floor; multi-offset/dma_gather/ap_gather forms all measured slower).
    alpha_d for the tile's 128 dst nodes comes from one [P,1] gather on
    ad_tab and is broadcast to edges via bf16 PE transposes of the one-hot
    S (built by DVE is_equal vs an iota). ex = exp(leakyrelu(as+ad)),
    msgs = [ex*h | ex] bf16, psum += S_j.T @ msgs_j aggregates weighted
    sums + softmax denominators. Normalize, ELU, PE-transpose, z @ W2 in
    bf16; y accumulated in SBUF and written once.
"""

import os
import sys

import numpy as np

if "/opt/trn_rl_repo" not in sys.path:
    sys.path.insert(0, "/opt/trn_rl_repo")

N_NODES = 50000
N_EDGES = 800000
F_IN = 128
HEADS = 4
HIDDEN = 32
F_OUT = 64
NEG = 0.2
N_CORES = 8
P = 128
FE = F_IN + 2 * HEADS    # 136 phase-0 psum cols: h | alpha_s | alpha_d
FM = F_IN + HEADS        # 132 message cols: ex*h | ex
EB = FE                  # bf16 elements per htab row (272 B)
AS_OFF = 64              # f32 element offset of alpha_s within a row
NODES_PER_CORE = N_NODES // N_CORES          # 6250
T_TILES = (NODES_PER_CORE + P - 1) // P      # 49
NPAD = T_TILES * 8 * P                       # 50176
CH = 14                                      # phase-0 node tiles per chunk

def _prep(edge_index, W, a_src, a_dst):
    """CPU-side: extended weights; edges sorted by (core, tile, src) and
    packed into [P, K] slot layout per destination tile."""
    src = np.asarray(edge_index[0]).astype(np.int64)
    dst = np.asarray(edge_index[1]).astype(np.int64)

    A_s = np.zeros((F_IN, HEADS), np.float32)
    A_d = np.zeros((F_IN, HEADS), np.float32)
    for h in range(HEADS):
        A_s[h * HIDDEN:(h + 1) * HIDDEN, h] = a_src[h]
        A_d[h * HIDDEN:(h + 1) * HIDDEN, h] = a_dst[h]
    W_ext = np.concatenate([W, W @ A_s, W @ A_d], axis=1).astype(np.float32)

    core_of = dst // NODES_PER_CORE
    # Bin-pack each core's dst nodes into T_TILES groups of exactly P nodes,
    # balancing edge counts (LPT): per-core edges avg NODES... ~100k/49 ~= 2041
    # < 2048, so nearly every group fits 16 slots of 128 edges.
    node_group = np.zeros((N_CORES, NODES_PER_CORE), np.int32)
    node_pos = np.zeros((N_CORES, NODES_PER_CORE), np.int32)
    group_nodes = np.zeros((N_CORES, T_TILES, P), np.int64)
    pad_from = np.zeros((N_CORES, T_TILES), np.int32)
    for c in range(N_CORES):
        lo, hi = c * NODES_PER_CORE, (c + 1) * NODES_PER_CORE
        deg = np.bincount(dst[core_of == c] - lo, minlength=NODES_PER_CORE)
        order_n = np.argsort(-deg, kind="stable")
        # serpentine deal for near-equal loads with equal node counts
        bins = [[] for _ in range(T_TILES)]
        bi, step = 0, 1
        for ln in order_n:
            while len(bins[bi]) >= P:
                bi += step
                if bi in (-1, T_TILES):
                    step = -step
                    bi += step
            bins[bi].append(ln)
            bi += step
            if bi in (-1, T_TILES):
                step = -step
                bi += step
        load = np.array([int(deg[b].sum()) for b in bins])
        # repair: push overflow (>16 slots) into the single heaviest bin by
        # swapping its light nodes for other bins' heavy nodes
        CAP = 16 * P
        b0 = int(np.argmax(load))
        for b in range(T_TILES):
            if b == b0:
                continue
            while load[b] > CAP:
                hi_i = max(range(len(bins[b])), key=lambda i: deg[bins[b][i]])
                lo_i = min(range(len(bins[b0])),
                           key=lambda i: deg[bins[b0][i]])
                nh, nl = bins[b][hi_i], bins[b0][lo_i]
                if deg[nh] <= deg[nl]:
                    break
                bins[b][hi_i], bins[b0][lo_i] = nl, nh
                load[b] += deg[nl] - deg[nh]
                load[b0] += deg[nh] - deg[nl]
        nfill = np.zeros(T_TILES, np.int32)
        for b in range(T_TILES):
            for pos, ln in enumerate(bins[b]):
                node_group[c, ln] = b
                node_pos[c, ln] = pos
                group_nodes[c, b, pos] = lo + ln
            nfill[b] = len(bins[b])
        # pad slots (tiles of the last 22 dummies) already absorbed: every
        # bin has exactly P real nodes (NODES_PER_CORE=6250 < 49*128=6272)
        for b in np.flatnonzero(nfill < P):
            group_nodes[c, b, nfill[b]:] = lo  # harmless duplicate for adt
        pad_from[c] = nfill

    group_all = (core_of * T_TILES
                 + node_group[core_of, dst - core_of * NODES_PER_CORE])
    order = np.lexsort((src, group_all))
    src_s, dst_s, group_s = src[order], dst[order], group_all[order]

    NG = N_CORES * T_TILES
    gs = np.searchsorted(group_s, np.arange(NG))
    ge = np.searchsorted(group_s, np.arange(NG), side="right")
    cnt = (ge - gs).reshape(N_CORES, T_TILES)
    # Sort each core's groups by edge count (desc): loop position t then only
    # needs k_list[t] = max over cores of the t-th largest slot count.
    perm = np.argsort(-cnt, axis=1)
    cnt_sorted = np.take_along_axis(cnt, perm, axis=1)
    k_list = [max(1, int(np.max((cnt_sorted[:, t] + P - 1) // P)))
              for t in range(T_TILES)]
    K = max(k_list)

    src32 = np.zeros((N_CORES, T_TILES, P, K), np.int32)
    d_local = np.full((N_CORES, T_TILES, P, K), -1.0, np.float32)
    dst_nodes = np.zeros((N_CORES, T_TILES, P, 1), np.int32)
    node_order = np.zeros((N_CORES, T_TILES, P), np.int64)
    # dlocT[c, t, j*P + e] = d_local[c, t, e, j] (edge-slot-major, for STt)
    for c in range(N_CORES):
        for tp in range(T_TILES):
            t = perm[c, tp]
            g = c * T_TILES + t
            s, e = gs[g], ge[g]
            n = e - s
            i = np.arange(n)
            dl = node_pos[c, dst_s[s:e] - c * NODES_PER_CORE]
            src32[c, tp, i % P, i // P] = src_s[s:e]
            d_local[c, tp, i % P, i // P] = dl
            dst_nodes[c, tp, :, 0] = group_nodes[c, t]
            node_order[c, tp] = group_nodes[c, t]
            if pad_from[c, t] < P:
                node_order[c, tp, pad_from[c, t]:] = -1
    dlocT = np.ascontiguousarray(
        d_local.transpose(0, 1, 3, 2)).reshape(N_CORES, T_TILES, K * P)
    return W_ext, src32, d_local, dlocT, dst_nodes, node_order, k_list


def _build_module(k_list, bias_nz, b2_nz):
    K = max(k_list)
    import concourse.bass as bass
    import concourse.mybir as mybir
    import concourse.tile as tile
    from concourse import bacc
    from concourse.masks import make_identity

    f32 = mybir.dt.float32
    bf16 = mybir.dt.bfloat16
    i32 = mybir.dt.int32

    nc = bacc.Bacc("TRN2", target_bir_lowering=False, debug=False,
                   num_devices=N_CORES, dynamic_dma_scratch_size=32768)

    x_T = nc.dram_tensor("x_T", [P, NPAD], bf16, kind="ExternalInput")
    W_ext_d = nc.dram_tensor("W_ext", [P, FE], bf16, kind="ExternalInput")
    W2_d = nc.dram_tensor("W2", [P, F_OUT], bf16, kind="ExternalInput")
    s32_d = nc.dram_tensor("src32", [T_TILES, P, K], i32,
                           kind="ExternalInput")
    dstn_d = nc.dram_tensor("dst_nodes", [T_TILES, P, 1], i32,
                            kind="ExternalInput")
    dloc_d = nc.dram_tensor("d_local", [T_TILES, P, K], bf16,
                            kind="ExternalInput")
    dlocT_d = nc.dram_tensor("d_localT", [T_TILES, K * P], bf16,
                             kind="ExternalInput")
    if bias_nz:
        bias_d = nc.dram_tensor("bias_ext", [1, FE], bf16,
                                kind="ExternalInput")
    if b2_nz:
        b2_d = nc.dram_tensor("b2_row", [1, F_OUT], bf16,
                              kind="ExternalInput")
    y_d = nc.dram_tensor("y_out", [T_TILES * P, F_OUT], f32,
                         kind="ExternalOutput")
    htab = nc.dram_tensor("htab", [NPAD, EB], bf16, kind="Internal")
    ad_tab = nc.dram_tensor("ad_tab", [NPAD, HEADS], f32, kind="Internal")

    add = mybir.AluOpType.add
    mult = mybir.AluOpType.mult
    amax = mybir.AluOpType.max
    is_eq = mybir.AluOpType.is_equal
    Exp = mybir.ActivationFunctionType.Exp

    N_CHUNKS = NPAD // (CH * P)  # 28

    with tile.TileContext(nc) as tc:
        with tc.tile_pool(name="const", bufs=1) as constp:
            W_ext_sb = constp.tile([P, FE], bf16)
            nc.sync.dma_start(W_ext_sb[:], W_ext_d.ap())
            W2_sb = constp.tile([P, F_OUT], bf16)
            nc.sync.dma_start(W2_sb[:], W2_d.ap())
            iota_f = constp.tile([P, P], f32)
            nc.gpsimd.iota(iota_f[:], pattern=[[1, P]], base=0,
                           channel_multiplier=0,
                           allow_small_or_imprecise_dtypes=True)
            iota_sb = constp.tile([P, P], bf16)
            nc.vector.tensor_copy(iota_sb[:], iota_f[:])
            iotap_f = constp.tile([P, 1], f32)
            nc.gpsimd.iota(iotap_f[:], pattern=[[0, 1]], base=0,
                           channel_multiplier=1,
                           allow_small_or_imprecise_dtypes=True)
            iotap = constp.tile([P, 1], bf16)
            nc.vector.tensor_copy(iotap[:], iotap_f[:])
            ident_f = constp.tile([P, P], f32)
            make_identity(nc, ident_f[:])
            ident = constp.tile([P, P], bf16)
            nc.vector.tensor_copy(ident[:], ident_f[:])
            ones1 = constp.tile([1, P], bf16)
            nc.vector.memset(ones1[:], 1.0)
            s32_sb = constp.tile([P, T_TILES, K], i32)
            nc.sync.dma_start(s32_sb[:],
                              s32_d.ap().rearrange("t p k -> p t k"))
            dstn_sb = constp.tile([P, T_TILES], i32)
            nc.sync.dma_start(dstn_sb[:],
                              dstn_d.ap().rearrange("t p one -> p (t one)"))
            dloc_sb = constp.tile([P, T_TILES, K], bf16)
            nc.sync.dma_start(dloc_sb[:],
                              dloc_d.ap().rearrange("t p k -> p t k"))
            if bias_nz or b2_nz:
                ones_sb = constp.tile([1, P], bf16)
                nc.vector.memset(ones_sb[:], 1.0)
            if bias_nz:
                bias_sb = constp.tile([1, FE], bf16)
                nc.sync.dma_start(bias_sb[:], bias_d.ap())
            if b2_nz:
                b2_sb = constp.tile([1, F_OUT], bf16)
                nc.sync.dma_start(b2_sb[:], b2_d.ap())
            ad_acc = constp.tile([P, NPAD // P, HEADS], f32)
            y_acc = constp.tile([P, T_TILES, F_OUT], f32)

            # ---- phase 0: htab = [x@W_ext | as]; ad_acc = ad ----
            # 3 node-tiles share one PSUM bank so the PSUM->SBUF copies
            # amortize the DVE per-op overhead.
            with (
                tc.tile_pool(name="xt", bufs=3) as xtp,
                tc.tile_pool(name="hx", bufs=3) as hxp,
                tc.tile_pool(name="p0ps", bufs=4, space="PSUM") as p0ps,
            ):
                for c in range(N_CHUNKS):
                    xt = xtp.tile([P, CH * P], bf16)
                    nc.scalar.dma_start(
                        xt[:], x_T.ap()[:, c * CH * P:(c + 1) * CH * P])
                    hrow = hxp.tile([P, CH, EB], bf16, tag="hrow")
                    hrow_f32 = hrow[:].bitcast(f32)
                    for j0 in range(0, CH, 3):
                        nj = min(3, CH - j0)
                        ps = p0ps.tile([P, 3, FE], f32)
                        for j in range(j0, j0 + nj):
                            nc.tensor.matmul(
                                ps[:, j - j0, :],
                                lhsT=xt[:, j * P:(j + 1) * P],
                                rhs=W_ext_sb[:], start=True,
                                stop=not bias_nz)
                            if bias_nz:
                                nc.tensor.matmul(ps[:, j - j0, :],
                                                 lhsT=ones_sb[:],
                                                 rhs=bias_sb[:], start=False,
                                                 stop=True)
                        nc.vector.tensor_copy(
                            hrow[:, j0:j0 + nj, 0:F_IN],
                            ps[:, 0:nj, 0:F_IN])
                        nc.vector.tensor_copy(
                            hrow_f32[:, j0:j0 + nj, AS_OFF:AS_OFF + HEADS],
                            ps[:, 0:nj, F_IN:F_IN + HEADS])
                        nc.vector.tensor_copy(
                            ad_acc[:, c * CH + j0:c * CH + j0 + nj, :],
                            ps[:, 0:nj, F_IN + HEADS:FE])
                    rows = slice(c * CH * P, (c + 1) * CH * P)
                    nc.sync.dma_start(
                        htab.ap()[rows, :].rearrange("(t p) e -> p t e", p=P),
                        hrow[:])
                    if (c + 1) % 7 == 0:  # quarters: after chunks 6,13,20,27
                        q = slice((c - 6) * CH * P, (c + 1) * CH * P)
                        nc.sync.dma_start(
                            ad_tab.ap()[q, :].rearrange(
                                "(t p) e -> p t e", p=P),
                            ad_acc[:, (c - 6) * CH:(c + 1) * CH, :])

            # ---- phase 1: per destination tile ----
            BC = 512  # bcast-matmul chunk (one PSUM bank of f32)
            with (
                tc.tile_pool(name="g", bufs=6) as gp,
                tc.tile_pool(name="msgs", bufs=3) as mp,
                tc.tile_pool(name="S", bufs=2) as sp,
                tc.tile_pool(name="STt", bufs=2) as stp,
                tc.tile_pool(name="dlT", bufs=2) as dlp,
                tc.tile_pool(name="agg", bufs=2, space="PSUM") as aggp,
                tc.tile_pool(name="bcps", bufs=2, space="PSUM") as bcpsp,
                tc.tile_pool(name="adps", bufs=2, space="PSUM") as adpsp,
                tc.tile_pool(name="small", bufs=4) as smallp,
                tc.tile_pool(name="tr", bufs=1, space="PSUM") as trp,
                tc.tile_pool(name="yps", bufs=1, space="PSUM") as ypsp,
            ):
                for t in range(T_TILES):
                    Kt = k_list[t]
                    adt = smallp.tile([P, HEADS], f32, tag="adt")
                    nc.gpsimd.indirect_dma_start(
                        out=adt[:], out_offset=None, in_=ad_tab.ap(),
                        in_offset=bass.IndirectOffsetOnAxis(
                            ap=dstn_sb[:, t:t + 1], axis=0))
                    g = gp.tile([P, Kt, EB], bf16, tag="g")
                    for j in range(Kt):
                        nc.gpsimd.indirect_dma_start(
                            out=g[:, j, :], out_offset=None,
                            in_=htab.ap(),
                            in_offset=bass.IndirectOffsetOnAxis(
                                ap=s32_sb[:, t, j:j + 1], axis=0))
                    adtb = smallp.tile([P, HEADS], bf16, tag="adtb")
                    nc.vector.tensor_copy(adtb[:], adt[:])

                    # one-hot scatter matrix S[p=e, (k, d)] in bf16
                    S = sp.tile([P, Kt, P], bf16, tag="S")
                    nc.vector.tensor_tensor(
                        out=S[:],
                        in0=iota_sb[:].unsqueeze(1).to_broadcast([P, Kt, P]),
                        in1=dloc_sb[:, t, 0:Kt].unsqueeze(2).to_broadcast(
                            [P, Kt, P]),
                        op=is_eq)
                    # STt[d, (j e)] = (d == dloc[e, j]) built directly:
                    # dlocT row broadcast across partitions via PE, then one
                    # is_equal against the partition-index iota.
                    dlT_row = dlp.tile([1, Kt * P], bf16, tag="dlr")
                    nc.sync.dma_start(dlT_row[:],
                                      dlocT_d.ap()[t:t + 1, 0:Kt * P])
                    dlT = dlp.tile([P, Kt * P], bf16, tag="dlT")
                    for q0 in range(0, Kt * P, BC):
                        qn = min(BC, Kt * P - q0)
                        bps = bcpsp.tile([P, BC], f32)
                        nc.tensor.matmul(bps[:, 0:qn], lhsT=ones1[:],
                                         rhs=dlT_row[:, q0:q0 + qn],
                                         start=True, stop=True)
                        nc.vector.tensor_copy(dlT[:, q0:q0 + qn],
                                              bps[:, 0:qn])
                    STt = stp.tile([P, Kt * P], bf16, tag="STt")
                    nc.vector.tensor_tensor(
                        out=STt[:],
                        in0=iotap[:].to_broadcast([P, Kt * P]),
                        in1=dlT[:], op=is_eq)
                    # alpha_d per edge: adps[:, j*4:(j+1)*4] = STt_j.T @ adtb
                    adps = adpsp.tile([P, Kt * HEADS], f32, tag="adps")
                    for j in range(Kt):
                        nc.tensor.matmul(
                            adps[:, j * HEADS:(j + 1) * HEADS],
                            lhsT=STt[:, j * P:(j + 1) * P], rhs=adtb[:],
                            start=True, stop=True)

                    # ex = exp(leakyrelu(alpha_s + alpha_d)) per edge
                    gf = g[:].bitcast(f32)   # [P, Kt, 68]
                    ex = smallp.tile([P, Kt, HEADS], f32, tag="ex")
                    nc.vector.tensor_tensor(
                        out=ex[:], in0=gf[:, :, AS_OFF:AS_OFF + HEADS],
                        in1=adps[:].rearrange("p (k h) -> p k h", k=Kt),
                        op=add)
                    nc.vector.scalar_tensor_tensor(
                        out=ex[:], in0=ex[:], scalar=NEG, in1=ex[:],
                        op0=mult, op1=amax)
                    exb = smallp.tile([P, Kt, HEADS], bf16, tag="exb")
                    nc.scalar.activation(out=exb[:], in_=ex[:], func=Exp)

                    msgs = mp.tile([P, Kt, FM], bf16, tag="msgs")
                    nc.vector.tensor_tensor(
                        out=msgs[:, :, 0:F_IN].rearrange(
                            "p k (h f) -> p k h f", h=HEADS),
                        in0=g[:, :, 0:F_IN].rearrange(
                            "p k (h f) -> p k h f", h=HEADS),
                        in1=exb[:].unsqueeze(3).to_broadcast(
                            [P, Kt, HEADS, HIDDEN]),
                        op=mult)
                    nc.vector.tensor_copy(msgs[:, :, F_IN:FM], exb[:])

                    ps = aggp.tile([P, FM], f32)
                    for j in range(Kt):
                        nc.tensor.matmul(ps[:], lhsT=S[:, j, :],
                                         rhs=msgs[:, j, :],
                                         start=(j == 0), stop=(j == K - 1))

                    rec = smallp.tile([P, HEADS], f32, tag="rec")
                    nc.vector.tensor_scalar_add(out=rec[:],
                                                in0=ps[:, F_IN:FM],
                                                scalar1=1e-16)
                    nc.vector.reciprocal(rec[:], rec[:])
                    zn = smallp.tile([P, F_IN], f32, tag="zn")
                    nc.vector.tensor_tensor(
                        out=zn[:].rearrange("p (h f) -> p h f", h=HEADS),
                        in0=ps[:, 0:F_IN].rearrange("p (h f) -> p h f",
                                                    h=HEADS),
                        in1=rec[:].unsqueeze(2).to_broadcast(
                            [P, HEADS, HIDDEN]),
                        op=mult)
                    # ELU(z) = max(z, exp(min(z,0)) - 1)
                    tmp = smallp.tile([P, F_IN], f32, tag="tmp")
                    nc.vector.tensor_scalar_min(out=tmp[:], in0=zn[:],
                                                scalar1=0.0)
                    nc.scalar.activation(out=tmp[:], in_=tmp[:], func=Exp)
                    znb = smallp.tile([P, F_IN], bf16, tag="znb")
                    nc.vector.scalar_tensor_tensor(
                        out=znb[:], in0=tmp[:], scalar=-1.0, in1=zn[:],
                        op0=add, op1=amax)

                    pt = trp.tile([P, P], bf16, tag="pt")
                    nc.tensor.transpose(out=pt[:], in_=znb[:],
                                        identity=ident[:])
                    znT = smallp.tile([P, P], bf16, tag="znT")
                    nc.vector.tensor_copy(znT[:], pt[:])
                    yp = ypsp.tile([P, F_OUT], f32, tag="yp")
                    nc.tensor.matmul(yp[:], lhsT=znT[:], rhs=W2_sb[:],
                                     start=True, stop=not b2_nz)
                    if b2_nz:
                        nc.tensor.matmul(yp[:], lhsT=ones_sb[:], rhs=b2_sb[:],
                                         start=False, stop=True)
                    nc.vector.tensor_copy(y_acc[:, t, :], yp[:])
            nc.sync.dma_start(
                y_d.ap().rearrange("(t p) f -> p t f", p=P), y_acc[:])

    nc.compile()
    return nc


_MODULE_CACHE = {}


def _get_module(k_list, bias_nz, b2_nz):
    key = (tuple(k_list), bias_nz, b2_nz)
    if key not in _MODULE_CACHE:
        _MODULE_CACHE[key] = _build_module(k_list, bias_nz, b2_nz)
    return _MODULE_CACHE[key]


def _ensure_ntff_hook():
    """The axon NTFF profile hook lives in antenv.axon_hooks, which this
    image's antenv package lacks; shim it so trace=True works."""
    try:
        import antenv.axon_hooks  # noqa: F401
        return
    except ImportError:
        pass
    import types

    import antenv

    mod = types.ModuleType("antenv.axon_hooks")
    holder = {"h": None}
    mod.set_axon_ntff_profile_hook = lambda h: holder.__setitem__("h", h)
    mod.get_axon_ntff_profile_hook = lambda: holder["h"]
    try:
        from trn_agent_boot.trn_boot import _ntff_profile_via_ctypes
        holder["h"] = _ntff_profile_via_ctypes("/opt/axon/libaxon_pjrt.so")
    except Exception:
        pass
    sys.modules["antenv.axon_hooks"] = mod
    antenv.axon_hooks = mod


def kernel(x, edge_index, edge_weight, W, a_src, a_dst, bias, W2, b2,
           _trace=False):
    import ml_dtypes
    from concourse.bass_utils import run_bass_kernel_spmd

    bf = ml_dtypes.bfloat16
    if _trace:
        _ensure_ntff_hook()

    x = np.asarray(x, np.float32)
    W = np.asarray(W, np.float32)
    a_src = np.asarray(a_src, np.float32)
    a_dst = np.asarray(a_dst, np.float32)
    bias = np.asarray(bias, np.float32)
    W2 = np.asarray(W2, np.float32)
    b2 = np.asarray(b2, np.float32)

    W_ext, src32, d_local, dlocT, dst_nodes, node_order, k_list = _prep(
        edge_index, W, a_src, a_dst)

    bias_nz = bool(np.any(bias))
    b2_nz = bool(np.any(b2))
    nc = _get_module(k_list, bias_nz, b2_nz)

    x_T = np.zeros((P, NPAD), bf)
    x_T[:, :N_NODES] = x.T.astype(bf)

    in_maps = []
    for c in range(N_CORES):
        m = {
            "x_T": x_T,
            "W_ext": W_ext.astype(bf),
            "W2": W2.astype(bf),
            "src32": np.ascontiguousarray(src32[c]),
            "dst_nodes": np.ascontiguousarray(dst_nodes[c]),
            "d_local": np.ascontiguousarray(d_local[c].astype(bf)),
            "d_localT": np.ascontiguousarray(dlocT[c].astype(bf)),
        }
        if bias_nz:
            be = np.zeros((1, FE), np.float32)
            be[0, :F_IN] = bias
            m["bias_ext"] = be.astype(bf)
        if b2_nz:
            m["b2_row"] = b2.reshape(1, F_OUT).astype(bf)
        in_maps.append(m)

    res = run_bass_kernel_spmd(nc, in_maps, core_ids=list(range(N_CORES)),
                               trace=_trace)
    out = np.zeros((N_NODES, F_OUT), np.float32)
    for c in range(N_CORES):
        y = res.results[c]["y_out"].reshape(T_TILES * P, F_OUT)
        idx = node_order[c].reshape(-1)
        valid = idx >= 0
        out[idx[valid]] = y[valid]
    if _trace:
        kernel.last_results = res
    return out


# revision 6
# speedup vs baseline: 1.1141x; 1.0002x over previous
"""GAT layer (nn_GAT_40037685133531) as a Trainium2 Bass kernel on 8 NeuronCores.

Strategy (graph/data parallel, no collectives):
  - Destination nodes sharded 8 ways (6250 per core), then bin-packed into
    49 balanced 128-node groups per core (serpentine deal + overflow
    concentrated into one group) so 48 of 49 groups need exactly 16
    edge slots of 128; per-position slot counts are the max over cores.
  - Phase 0 (replicated, bf16): h_ext = x @ [W | W@A_s | W@A_d] -> htab
    [NPAD, 136] bf16 rows = [h bf16 x128 | alpha_s f32 x4] (272B); alpha_d
    accumulated in SBUF and written once to ad_tab [NPAD, 4] f32.
  - Phase 1 per tile: K x [P,1]-offset indirect DMAs gather the edge rows
    by src (measured: ~1.1us/op fixed issue cost on gpsimd is the kernel
    floor; multi-offset/dma_gather/ap_gather forms all measured slower).
    alpha_d for the tile's 128 dst nodes comes from one [P,1] gather on
    ad_tab and is broadcast to edges via bf16 PE transposes of the one-hot
    S (built by DVE is_equal vs an iota). ex = exp(leakyrelu(as+ad)),
    msgs = [ex*h | ex] bf16, psum += S_j.T @ msgs_j aggregates weighted
    sums + softmax denominators. Normalize, ELU, PE-transpose, z @ W2 in
    bf16; y accumulated in SBUF and written once.
"""

import os
import sys

import numpy as np

if "/opt/trn_rl_repo" not in sys.path:
    sys.path.insert(0, "/opt/trn_rl_repo")

N_NODES = 50000
N_EDGES = 800000
F_IN = 128
HEADS = 4
HIDDEN = 32
F_OUT = 64
NEG = 0.2
N_CORES = 8
P = 128
FE = F_IN + 2 * HEADS    # 136 phase-0 psum cols: h | alpha_s | alpha_d
FM = F_IN + HEADS        # 132 message cols: ex*h | ex
EB = FE                  # bf16 elements per htab row (272 B)
AS_OFF = 64              # f32 element offset of alpha_s within a row
NODES_PER_CORE = N_NODES // N_CORES          # 6250
T_TILES = (NODES_PER_CORE + P - 1) // P      # 49
NPAD = T_TILES * 8 * P                       # 50176
CH = 14                                      # phase-0 node tiles per chunk

def _prep(edge_index, W, a_src, a_dst):
    """CPU-side: extended weights; edges sorted by (core, tile, src) and
    packed into [P, K] slot layout per destination tile."""
    src = np.asarray(edge_index[0]).astype(np.int64)
    dst = np.asarray(edge_index[1]).astype(np.int64)

    A_s = np.zeros((F_IN, HEADS), np.float32)
    A_d = np.zeros((F_IN, HEADS), np.float32)
    for h in range(HEADS):
        A_s[h * HIDDEN:(h + 1) * HIDDEN, h] = a_src[h]
        A_d[h * HIDDEN:(h + 1) * HIDDEN, h] = a_dst[h]
    W_ext = np.concatenate([W, W @ A_s, W @ A_d], axis=1).astype(np.float32)

    core_of = dst // NODES_PER_CORE
    # Bin-pack each core's dst nodes into T_TILES groups of exactly P nodes,
    # balancing edge counts (LPT): per-core edges avg NODES... ~100k/49 ~= 2041
    # < 2048, so nearly every group fits 16 slots of 128 edges.
    node_group = np.zeros((N_CORES, NODES_PER_CORE), np.int32)
    node_pos = np.zeros((N_CORES, NODES_PER_CORE), np.int32)
    group_nodes = np.zeros((N_CORES, T_TILES, P), np.int64)
    pad_from = np.zeros((N_CORES, T_TILES), np.int32)
    for c in range(N_CORES):
        lo, hi = c * NODES_PER_CORE, (c + 1) * NODES_PER_CORE
        deg = np.bincount(dst[core_of == c] - lo, minlength=NODES_PER_CORE)
        order_n = np.argsort(-deg, kind="stable")
        # serpentine deal for near-equal loads with equal node counts
        bins = [[] for _ in range(T_TILES)]
        bi, step = 0, 1
        for ln in order_n:
            while len(bins[bi]) >= P:
                bi += step
                if bi in (-1, T_TILES):
                    step = -step
                    bi += step
            bins[bi].append(ln)
            bi += step
            if bi in (-1, T_TILES):
                step = -step
                bi += step
        load = np.array([int(deg[b].sum()) for b in bins])
        # repair: push overflow (>16 slots) into the single heaviest bin by
        # swapping its light nodes for other bins' heavy nodes
        CAP = 16 * P
        b0 = int(np.argmax(load))
        for b in range(T_TILES):
            if b == b0:
                continue
            while load[b] > CAP:
                hi_i = max(range(len(bins[b])), key=lambda i: deg[bins[b][i]])
                lo_i = min(range(len(bins[b0])),
                           key=lambda i: deg[bins[b0][i]])
                nh, nl = bins[b][hi_i], bins[b0][lo_i]
                if deg[nh] <= deg[nl]:
                    break
                bins[b][hi_i], bins[b0][lo_i] = nl, nh
                load[b] += deg[nl] - deg[nh]
                load[b0] += deg[nh] - deg[nl]
        nfill = np.zeros(T_TILES, np.int32)
        for b in range(T_TILES):
            for pos, ln in enumerate(bins[b]):
                node_group[c, ln] = b
                node_pos[c, ln] = pos
                group_nodes[c, b, pos] = lo + ln
            nfill[b] = len(bins[b])
        # pad slots (tiles of the last 22 dummies) already absorbed: every
        # bin has exactly P real nodes (NODES_PER_CORE=6250 < 49*128=6272)
        for b in np.flatnonzero(nfill < P):
            group_nodes[c, b, nfill[b]:] = lo  # harmless duplicate for adt
        pad_from[c] = nfill

    group_all = (core_of * T_TILES
                 + node_group[core_of, dst - core_of * NODES_PER_CORE])
    order = np.lexsort((src, group_all))
    src_s, dst_s, group_s = src[order], dst[order], group_all[order]

    NG = N_CORES * T_TILES
    gs = np.searchsorted(group_s, np.arange(NG))
    ge = np.searchsorted(group_s, np.arange(NG), side="right")
    cnt = (ge - gs).reshape(N_CORES, T_TILES)
    # Sort each core's groups by edge count (desc): loop position t then only
    # needs k_list[t] = max over cores of the t-th largest slot count.
    perm = np.argsort(-cnt, axis=1)
    cnt_sorted = np.take_along_axis(cnt, perm, axis=1)
    k_list = [max(1, int(np.max((cnt_sorted[:, t] + P - 1) // P)))
              for t in range(T_TILES)]
    K = max(k_list)

    src32 = np.zeros((N_CORES, T_TILES, P, K), np.int32)
    d_local = np.full((N_CORES, T_TILES, P, K), -1.0, np.float32)
    dst_nodes = np.zeros((N_CORES, T_TILES, P, 1), np.int32)
    node_order = np.zeros((N_CORES, T_TILES, P), np.int64)
    # dlocT[c, t, j*P + e] = d_local[c, t, e, j] (edge-slot-major, for STt)
    for c in range(N_CORES):
        for tp in range(T_TILES):
            t = perm[c, tp]
            g = c * T_TILES + t
            s, e = gs[g], ge[g]
            n = e - s
            i = np.arange(n)
            dl = node_pos[c, dst_s[s:e] - c * NODES_PER_CORE]
            src32[c, tp, i % P, i // P] = src_s[s:e]
            d_local[c, tp, i % P, i // P] = dl
            dst_nodes[c, tp, :, 0] = group_nodes[c, t]
            node_order[c, tp] = group_nodes[c, t]
            if pad_from[c, t] < P:
                node_order[c, tp, pad_from[c, t]:] = -1
    dlocT = np.ascontiguousarray(
        d_local.transpose(0, 1, 3, 2)).reshape(N_CORES, T_TILES, K * P)
    # table rows are laid out partition-major (row = (n%128)*392 + n//128)
    # so phase-0 writes are contiguous per partition; bake the permutation
    # into the gather offsets.
    src32 = ((src32 % P) * (NPAD // P) + src32 // P).astype(np.int32)
    dst_nodes = ((dst_nodes % P) * (NPAD // P)
                 + dst_nodes // P).astype(np.int32)
    return W_ext, src32, d_local, dlocT, dst_nodes, node_order, k_list


def _build_module(k_list, bias_nz, b2_nz):
    K = max(k_list)
    import concourse.bass as bass
    import concourse.mybir as mybir
    import concourse.tile as tile
    from concourse import bacc
    from concourse.masks import make_identity

    f32 = mybir.dt.float32
    bf16 = mybir.dt.bfloat16
    i32 = mybir.dt.int32

    nc = bacc.Bacc("TRN2", target_bir_lowering=False, debug=False,
                   num_devices=N_CORES, dynamic_dma_scratch_size=32768)

    x_T = nc.dram_tensor("x_T", [P, NPAD], bf16, kind="ExternalInput")
    W_ext_d = nc.dram_tensor("W_ext", [P, FE], bf16, kind="ExternalInput")
    W2_d = nc.dram_tensor("W2", [P, F_OUT], bf16, kind="ExternalInput")
    s32_d = nc.dram_tensor("src32", [T_TILES, P, K], i32,
                           kind="ExternalInput")
    dstn_d = nc.dram_tensor("dst_nodes", [T_TILES, P, 1], i32,
                            kind="ExternalInput")
    dloc_d = nc.dram_tensor("d_local", [T_TILES, P, K], bf16,
                            kind="ExternalInput")
    dlocT_d = nc.dram_tensor("d_localT", [T_TILES, K * P], bf16,
                             kind="ExternalInput")
    if bias_nz:
        bias_d = nc.dram_tensor("bias_ext", [1, FE], bf16,
                                kind="ExternalInput")
    if b2_nz:
        b2_d = nc.dram_tensor("b2_row", [1, F_OUT], bf16,
                              kind="ExternalInput")
    y_d = nc.dram_tensor("y_out", [T_TILES * P, F_OUT], f32,
                         kind="ExternalOutput")
    htab = nc.dram_tensor("htab", [NPAD, EB], bf16, kind="Internal")
    ad_tab = nc.dram_tensor("ad_tab", [NPAD, HEADS], f32, kind="Internal")

    add = mybir.AluOpType.add
    mult = mybir.AluOpType.mult
    amax = mybir.AluOpType.max
    is_eq = mybir.AluOpType.is_equal
    Exp = mybir.ActivationFunctionType.Exp

    N_CHUNKS = NPAD // (CH * P)  # 28

    with tile.TileContext(nc) as tc:
        with tc.tile_pool(name="const", bufs=1) as constp:
            W_ext_sb = constp.tile([P, FE], bf16)
            nc.sync.dma_start(W_ext_sb[:], W_ext_d.ap())
            W2_sb = constp.tile([P, F_OUT], bf16)
            nc.sync.dma_start(W2_sb[:], W2_d.ap())
            iota_f = constp.tile([P, P], f32)
            nc.gpsimd.iota(iota_f[:], pattern=[[1, P]], base=0,
                           channel_multiplier=0,
                           allow_small_or_imprecise_dtypes=True)
            iota_sb = constp.tile([P, P], bf16)
            nc.vector.tensor_copy(iota_sb[:], iota_f[:])
            iotap_f = constp.tile([P, 1], f32)
            nc.gpsimd.iota(iotap_f[:], pattern=[[0, 1]], base=0,
                           channel_multiplier=1,
                           allow_small_or_imprecise_dtypes=True)
            iotap = constp.tile([P, 1], bf16)
            nc.vector.tensor_copy(iotap[:], iotap_f[:])
            ident_f = constp.tile([P, P], f32)
            make_identity(nc, ident_f[:])
            ident = constp.tile([P, P], bf16)
            nc.vector.tensor_copy(ident[:], ident_f[:])
            ones1 = constp.tile([1, P], bf16)
            nc.vector.memset(ones1[:], 1.0)
            s32_sb = constp.tile([P, T_TILES, K], i32)
            nc.sync.dma_start(s32_sb[:],
                              s32_d.ap().rearrange("t p k -> p t k"))
            dstn_sb = constp.tile([P, T_TILES], i32)
            nc.sync.dma_start(dstn_sb[:],
                              dstn_d.ap().rearrange("t p one -> p (t one)"))
            dloc_sb = constp.tile([P, T_TILES, K], bf16)
            nc.sync.dma_start(dloc_sb[:],
                              dloc_d.ap().rearrange("t p k -> p t k"))
            if bias_nz or b2_nz:
                ones_sb = constp.tile([1, P], bf16)
                nc.vector.memset(ones_sb[:], 1.0)
            if bias_nz:
                bias_sb = constp.tile([1, FE], bf16)
                nc.sync.dma_start(bias_sb[:], bias_d.ap())
            if b2_nz:
                b2_sb = constp.tile([1, F_OUT], bf16)
                nc.sync.dma_start(b2_sb[:], b2_d.ap())
            ad_acc = constp.tile([P, NPAD // P, HEADS], f32)
            y_acc = constp.tile([P, T_TILES, F_OUT], f32)

            # ---- phase 0: htab = [x@W_ext | as]; ad_acc = ad ----
            # 3 node-tiles share one PSUM bank so the PSUM->SBUF copies
            # amortize the DVE per-op overhead.
            with (
                tc.tile_pool(name="xt", bufs=3) as xtp,
                tc.tile_pool(name="hx", bufs=3) as hxp,
                tc.tile_pool(name="p0ps", bufs=4, space="PSUM") as p0ps,
            ):
                for c in range(N_CHUNKS):
                    xt = xtp.tile([P, CH * P], bf16)
                    nc.scalar.dma_start(
                        xt[:], x_T.ap()[:, c * CH * P:(c + 1) * CH * P])
                    hrow = hxp.tile([P, CH, EB], bf16, tag="hrow")
                    hrow_f32 = hrow[:].bitcast(f32)
                    for j0 in range(0, CH, 3):
                        nj = min(3, CH - j0)
                        ps = p0ps.tile([P, 3, FE], f32)
                        for j in range(j0, j0 + nj):
                            nc.tensor.matmul(
                                ps[:, j - j0, :],
                                lhsT=xt[:, j * P:(j + 1) * P],
                                rhs=W_ext_sb[:], start=True,
                                stop=not bias_nz)
                            if bias_nz:
                                nc.tensor.matmul(ps[:, j - j0, :],
                                                 lhsT=ones_sb[:],
                                                 rhs=bias_sb[:], start=False,
                                                 stop=True)
                        nc.vector.tensor_copy(
                            hrow[:, j0:j0 + nj, 0:F_IN],
                            ps[:, 0:nj, 0:F_IN])
                        nc.vector.tensor_copy(
                            hrow_f32[:, j0:j0 + nj, AS_OFF:AS_OFF + HEADS],
                            ps[:, 0:nj, F_IN:F_IN + HEADS])
                        nc.vector.tensor_copy(
                            ad_acc[:, c * CH + j0:c * CH + j0 + nj, :],
                            ps[:, 0:nj, F_IN + HEADS:FE])
                    htab_pt = htab.ap().rearrange("(p t) e -> p t e", p=P)
                    nc.sync.dma_start(
                        htab_pt[:, c * CH:(c + 1) * CH, :], hrow[:])
                    if (c + 1) % 7 == 0:  # quarters: after chunks 6,13,20,27
                        ad_pt = ad_tab.ap().rearrange("(p t) e -> p t e", p=P)
                        nc.sync.dma_start(
                            ad_pt[:, (c - 6) * CH:(c + 1) * CH, :],
                            ad_acc[:, (c - 6) * CH:(c + 1) * CH, :])

            # ---- phase 1: per destination tile ----
            BC = 512  # bcast-matmul chunk (one PSUM bank of f32)
            with (
                tc.tile_pool(name="g", bufs=6) as gp,
                tc.tile_pool(name="msgs", bufs=3) as mp,
                tc.tile_pool(name="S", bufs=2) as sp,
                tc.tile_pool(name="STt", bufs=2) as stp,
                tc.tile_pool(name="dlT", bufs=2) as dlp,
                tc.tile_pool(name="agg", bufs=2, space="PSUM") as aggp,
                tc.tile_pool(name="bcps", bufs=2, space="PSUM") as bcpsp,
                tc.tile_pool(name="adps", bufs=2, space="PSUM") as adpsp,
                tc.tile_pool(name="small", bufs=4) as smallp,
                tc.tile_pool(name="tr", bufs=1, space="PSUM") as trp,
                tc.tile_pool(name="yps", bufs=1, space="PSUM") as ypsp,
            ):
                for t in range(T_TILES):
                    Kt = k_list[t]
                    adt = smallp.tile([P, HEADS], f32, tag="adt")
                    nc.gpsimd.indirect_dma_start(
                        out=adt[:], out_offset=None, in_=ad_tab.ap(),
                        in_offset=bass.IndirectOffsetOnAxis(
                            ap=dstn_sb[:, t:t + 1], axis=0))
                    g = gp.tile([P, Kt, EB], bf16, tag="g")
                    for j in range(Kt):
                        nc.gpsimd.indirect_dma_start(
                            out=g[:, j, :], out_offset=None,
                            in_=htab.ap(),
                            in_offset=bass.IndirectOffsetOnAxis(
                                ap=s32_sb[:, t, j:j + 1], axis=0))
                    adtb = smallp.tile([P, HEADS], bf16, tag="adtb")
                    nc.vector.tensor_copy(adtb[:], adt[:])

                    # one-hot scatter matrix S[p=e, (k, d)] in bf16
                    S = sp.tile([P, Kt, P], bf16, tag="S")
                    nc.vector.tensor_tensor(
                        out=S[:],
                        in0=iota_sb[:].unsqueeze(1).to_broadcast([P, Kt, P]),
                        in1=dloc_sb[:, t, 0:Kt].unsqueeze(2).to_broadcast(
                            [P, Kt, P]),
                        op=is_eq)
                    # STt[d, (j e)] = (d == dloc[e, j]) built directly:
                    # dlocT row broadcast across partitions via PE, then one
                    # is_equal against the partition-index iota.
                    dlT_row = dlp.tile([1, Kt * P], bf16, tag="dlr")
                    nc.sync.dma_start(dlT_row[:],
                                      dlocT_d.ap()[t:t + 1, 0:Kt * P])
                    dlT = dlp.tile([P, Kt * P], bf16, tag="dlT")
                    for q0 in range(0, Kt * P, BC):
                        qn = min(BC, Kt * P - q0)
                        bps = bcpsp.tile([P, BC], f32)
                        nc.tensor.matmul(bps[:, 0:qn], lhsT=ones1[:],
                                         rhs=dlT_row[:, q0:q0 + qn],
                                         start=True, stop=True)
                        nc.vector.tensor_copy(dlT[:, q0:q0 + qn],
                                              bps[:, 0:qn])
                    STt = stp.tile([P, Kt * P], bf16, tag="STt")
                    nc.vector.tensor_tensor(
                        out=STt[:],
                        in0=iotap[:].to_broadcast([P, Kt * P]),
                        in1=dlT[:], op=is_eq)
                    # alpha_d per edge: adps[:, j*4:(j+1)*4] = STt_j.T @ adtb
                    adps = adpsp.tile([P, Kt * HEADS], f32, tag="adps")
                    for j in range(Kt):
                        nc.tensor.matmul(
                            adps[:, j * HEADS:(j + 1) * HEADS],
                            lhsT=STt[:, j * P:(j + 1) * P], rhs=adtb[:],
                            start=True, stop=True)

                    # ex = exp(leakyrelu(alpha_s + alpha_d)) per edge
                    gf = g[:].bitcast(f32)   # [P, Kt, 68]
                    ex = smallp.tile([P, Kt, HEADS], f32, tag="ex")
                    nc.vector.tensor_tensor(
                        out=ex[:], in0=gf[:, :, AS_OFF:AS_OFF + HEADS],
                        in1=adps[:].rearrange("p (k h) -> p k h", k=Kt),
                        op=add)
                    nc.vector.scalar_tensor_tensor(
                        out=ex[:], in0=ex[:], scalar=NEG, in1=ex[:],
                        op0=mult, op1=amax)
                    exb = smallp.tile([P, Kt, HEADS], bf16, tag="exb")
                    nc.scalar.activation(out=exb[:], in_=ex[:], func=Exp)

                    msgs = mp.tile([P, Kt, FM], bf16, tag="msgs")
                    nc.vector.tensor_tensor(
                        out=msgs[:, :, 0:F_IN].rearrange(
                            "p k (h f) -> p k h f", h=HEADS),
                        in0=g[:, :, 0:F_IN].rearrange(
                            "p k (h f) -> p k h f", h=HEADS),
                        in1=exb[:].unsqueeze(3).to_broadcast(
                            [P, Kt, HEADS, HIDDEN]),
                        op=mult)
                    nc.vector.tensor_copy(msgs[:, :, F_IN:FM], exb[:])

                    ps = aggp.tile([P, FM], f32)
                    for j in range(Kt):
                        nc.tensor.matmul(ps[:], lhsT=S[:, j, :],
                                         rhs=msgs[:, j, :],
                                         start=(j == 0), stop=(j == K - 1))

                    rec = smallp.tile([P, HEADS], f32, tag="rec")
                    nc.vector.tensor_scalar_add(out=rec[:],
                                                in0=ps[:, F_IN:FM],
                                                scalar1=1e-16)
                    nc.vector.reciprocal(rec[:], rec[:])
                    zn = smallp.tile([P, F_IN], f32, tag="zn")
                    nc.vector.tensor_tensor(
                        out=zn[:].rearrange("p (h f) -> p h f", h=HEADS),
                        in0=ps[:, 0:F_IN].rearrange("p (h f) -> p h f",
                                                    h=HEADS),
                        in1=rec[:].unsqueeze(2).to_broadcast(
                            [P, HEADS, HIDDEN]),
                        op=mult)
                    # ELU(z) = max(z, exp(min(z,0)) - 1)
                    tmp = smallp.tile([P, F_IN], f32, tag="tmp")
                    nc.vector.tensor_scalar_min(out=tmp[:], in0=zn[:],
                                                scalar1=0.0)
                    nc.scalar.activation(out=tmp[:], in_=tmp[:], func=Exp)
                    znb = smallp.tile([P, F_IN], bf16, tag="znb")
                    nc.vector.scalar_tensor_tensor(
                        out=znb[:], in0=tmp[:], scalar=-1.0, in1=zn[:],
                        op0=add, op1=amax)

                    pt = trp.tile([P, P], bf16, tag="pt")
                    nc.tensor.transpose(out=pt[:], in_=znb[:],
                                        identity=ident[:])
                    znT = smallp.tile([P, P], bf16, tag="znT")
                    nc.vector.tensor_copy(znT[:], pt[:])
                    yp = ypsp.tile([P, F_OUT], f32, tag="yp")
                    nc.tensor.matmul(yp[:], lhsT=znT[:], rhs=W2_sb[:],
                                     start=True, stop=not b2_nz)
                    if b2_nz:
                        nc.tensor.matmul(yp[:], lhsT=ones_sb[:], rhs=b2_sb[:],
                                         start=False, stop=True)
                    nc.vector.tensor_copy(y_acc[:, t, :], yp[:])
            nc.sync.dma_start(
                y_d.ap().rearrange("(t p) f -> p t f", p=P), y_acc[:])

    nc.compile()
    return nc


_MODULE_CACHE = {}


def _get_module(k_list, bias_nz, b2_nz):
    key = (tuple(k_list), bias_nz, b2_nz)
    if key not in _MODULE_CACHE:
        _MODULE_CACHE[key] = _build_module(k_list, bias_nz, b2_nz)
    return _MODULE_CACHE[key]


def _ensure_ntff_hook():
    """The axon NTFF profile hook lives in antenv.axon_hooks, which this
    image's antenv package lacks; shim it so trace=True works."""
    try:
        import antenv.axon_hooks  # noqa: F401
        return
    except ImportError:
        pass
    import types

    import antenv

    mod = types.ModuleType("antenv.axon_hooks")
    holder = {"h": None}
    mod.set_axon_ntff_profile_hook = lambda h: holder.__setitem__("h", h)
    mod.get_axon_ntff_profile_hook = lambda: holder["h"]
    try:
        from trn_agent_boot.trn_boot import _ntff_profile_via_ctypes
        holder["h"] = _ntff_profile_via_ctypes("/opt/axon/libaxon_pjrt.so")
    except Exception:
        pass
    sys.modules["antenv.axon_hooks"] = mod
    antenv.axon_hooks = mod


def kernel(x, edge_index, edge_weight, W, a_src, a_dst, bias, W2, b2,
           _trace=False):
    import ml_dtypes
    from concourse.bass_utils import run_bass_kernel_spmd

    bf = ml_dtypes.bfloat16
    if _trace:
        _ensure_ntff_hook()

    x = np.asarray(x, np.float32)
    W = np.asarray(W, np.float32)
    a_src = np.asarray(a_src, np.float32)
    a_dst = np.asarray(a_dst, np.float32)
    bias = np.asarray(bias, np.float32)
    W2 = np.asarray(W2, np.float32)
    b2 = np.asarray(b2, np.float32)

    W_ext, src32, d_local, dlocT, dst_nodes, node_order, k_list = _prep(
        edge_index, W, a_src, a_dst)

    bias_nz = bool(np.any(bias))
    b2_nz = bool(np.any(b2))
    nc = _get_module(k_list, bias_nz, b2_nz)

    x_T = np.zeros((P, NPAD), bf)
    x_T[:, :N_NODES] = x.T.astype(bf)

    in_maps = []
    for c in range(N_CORES):
        m = {
            "x_T": x_T,
            "W_ext": W_ext.astype(bf),
            "W2": W2.astype(bf),
            "src32": np.ascontiguousarray(src32[c]),
            "dst_nodes": np.ascontiguousarray(dst_nodes[c]),
            "d_local": np.ascontiguousarray(d_local[c].astype(bf)),
            "d_localT": np.ascontiguousarray(dlocT[c].astype(bf)),
        }
        if bias_nz:
            be = np.zeros((1, FE), np.float32)
            be[0, :F_IN] = bias
            m["bias_ext"] = be.astype(bf)
        if b2_nz:
            m["b2_row"] = b2.reshape(1, F_OUT).astype(bf)
        in_maps.append(m)

    res = run_bass_kernel_spmd(nc, in_maps, core_ids=list(range(N_CORES)),
                               trace=_trace)
    out = np.zeros((N_NODES, F_OUT), np.float32)
    for c in range(N_CORES):
        y = res.results[c]["y_out"].reshape(T_TILES * P, F_OUT)
        idx = node_order[c].reshape(-1)
        valid = idx >= 0
        out[idx[valid]] = y[valid]
    if _trace:
        kernel.last_results = res
    return out


# revision 7
# speedup vs baseline: 1.1633x; 1.0442x over previous
"""GAT layer (nn_GAT_40037685133531) as a Trainium2 Bass kernel on 8 NeuronCores.

Strategy (graph/data parallel, no collectives):
  - Destination nodes sharded 8 ways (6250 per core), then bin-packed into
    49 balanced 128-node groups per core (serpentine deal + overflow
    concentrated into one group) so 48 of 49 groups need exactly 16
    edge slots of 128; per-position slot counts are the max over cores.
  - Phase 0 (replicated, bf16): h_ext = x @ [W | W@A_s | W@A_d] -> htab
    [NPAD, 136] bf16 rows = [h bf16 x128 | alpha_s f32 x4] (272B); alpha_d
    accumulated in SBUF and written once to ad_tab [NPAD, 4] f32.
  - Phase 1 per tile: K x [P,1]-offset indirect DMAs gather the edge rows
    by src (measured: ~1.1us/op fixed issue cost on gpsimd is the kernel
    floor; multi-offset/dma_gather/ap_gather forms all measured slower).
    alpha_d for the tile's 128 dst nodes comes from one [P,1] gather on
    ad_tab and is broadcast to edges via bf16 PE transposes of the one-hot
    S (built by DVE is_equal vs an iota). ex = exp(leakyrelu(as+ad)),
    msgs = [ex*h | ex] bf16, psum += S_j.T @ msgs_j aggregates weighted
    sums + softmax denominators. Normalize, ELU, PE-transpose, z @ W2 in
    bf16; y accumulated in SBUF and written once.
"""

import os
import sys

import numpy as np

if "/opt/trn_rl_repo" not in sys.path:
    sys.path.insert(0, "/opt/trn_rl_repo")

N_NODES = 50000
N_EDGES = 800000
F_IN = 128
HEADS = 4
HIDDEN = 32
F_OUT = 64
NEG = 0.2
N_CORES = 8
P = 128
FE = F_IN + 2 * HEADS    # 136 phase-0 psum cols: h | alpha_s | alpha_d
FM = F_IN + HEADS        # 132 message cols: ex*h | ex
EB = FE                  # bf16 elements per htab row (272 B)
AS_OFF = 64              # f32 element offset of alpha_s within a row
NODES_PER_CORE = N_NODES // N_CORES          # 6250
T_TILES = (NODES_PER_CORE + P - 1) // P      # 49
NPAD = T_TILES * 8 * P                       # 50176
CH = 14                                      # phase-0 node tiles per chunk

def _prep(edge_index, W, a_src, a_dst):
    """CPU-side: extended weights; edges sorted by (core, tile, src) and
    packed into [P, K] slot layout per destination tile."""
    src = np.asarray(edge_index[0]).astype(np.int64)
    dst = np.asarray(edge_index[1]).astype(np.int64)

    A_s = np.zeros((F_IN, HEADS), np.float32)
    A_d = np.zeros((F_IN, HEADS), np.float32)
    for h in range(HEADS):
        A_s[h * HIDDEN:(h + 1) * HIDDEN, h] = a_src[h]
        A_d[h * HIDDEN:(h + 1) * HIDDEN, h] = a_dst[h]
    W_ext = np.concatenate([W, W @ A_s, W @ A_d], axis=1).astype(np.float32)

    core_of = dst // NODES_PER_CORE
    # Bin-pack each core's dst nodes into T_TILES groups of exactly P nodes,
    # balancing edge counts (LPT): per-core edges avg NODES... ~100k/49 ~= 2041
    # < 2048, so nearly every group fits 16 slots of 128 edges.
    node_group = np.zeros((N_CORES, NODES_PER_CORE), np.int32)
    node_pos = np.zeros((N_CORES, NODES_PER_CORE), np.int32)
    group_nodes = np.zeros((N_CORES, T_TILES, P), np.int64)
    pad_from = np.zeros((N_CORES, T_TILES), np.int32)
    for c in range(N_CORES):
        lo, hi = c * NODES_PER_CORE, (c + 1) * NODES_PER_CORE
        deg = np.bincount(dst[core_of == c] - lo, minlength=NODES_PER_CORE)
        order_n = np.argsort(-deg, kind="stable")
        # serpentine deal for near-equal loads with equal node counts
        bins = [[] for _ in range(T_TILES)]
        bi, step = 0, 1
        for ln in order_n:
            while len(bins[bi]) >= P:
                bi += step
                if bi in (-1, T_TILES):
                    step = -step
                    bi += step
            bins[bi].append(ln)
            bi += step
            if bi in (-1, T_TILES):
                step = -step
                bi += step
        load = np.array([int(deg[b].sum()) for b in bins])
        # repair: push overflow (>16 slots) into the single heaviest bin by
        # swapping its light nodes for other bins' heavy nodes
        CAP = 16 * P
        b0 = int(np.argmax(load))
        for b in range(T_TILES):
            if b == b0:
                continue
            while load[b] > CAP:
                hi_i = max(range(len(bins[b])), key=lambda i: deg[bins[b][i]])
                lo_i = min(range(len(bins[b0])),
                           key=lambda i: deg[bins[b0][i]])
                nh, nl = bins[b][hi_i], bins[b0][lo_i]
                if deg[nh] <= deg[nl]:
                    break
                bins[b][hi_i], bins[b0][lo_i] = nl, nh
                load[b] += deg[nl] - deg[nh]
                load[b0] += deg[nh] - deg[nl]
        nfill = np.zeros(T_TILES, np.int32)
        for b in range(T_TILES):
            for pos, ln in enumerate(bins[b]):
                node_group[c, ln] = b
                node_pos[c, ln] = pos
                group_nodes[c, b, pos] = lo + ln
            nfill[b] = len(bins[b])
        # pad slots (tiles of the last 22 dummies) already absorbed: every
        # bin has exactly P real nodes (NODES_PER_CORE=6250 < 49*128=6272)
        for b in np.flatnonzero(nfill < P):
            group_nodes[c, b, nfill[b]:] = lo  # harmless duplicate for adt
        pad_from[c] = nfill

    group_all = (core_of * T_TILES
                 + node_group[core_of, dst - core_of * NODES_PER_CORE])
    order = np.lexsort((src, group_all))
    src_s, dst_s, group_s = src[order], dst[order], group_all[order]

    NG = N_CORES * T_TILES
    gs = np.searchsorted(group_s, np.arange(NG))
    ge = np.searchsorted(group_s, np.arange(NG), side="right")
    cnt = (ge - gs).reshape(N_CORES, T_TILES)
    # Sort each core's groups by edge count (desc): loop position t then only
    # needs k_list[t] = max over cores of the t-th largest slot count.
    perm = np.argsort(-cnt, axis=1)
    cnt_sorted = np.take_along_axis(cnt, perm, axis=1)
    k_list = [max(1, int(np.max((cnt_sorted[:, t] + P - 1) // P)))
              for t in range(T_TILES)]
    K = max(k_list)

    src32 = np.zeros((N_CORES, T_TILES, P, K), np.int32)
    d_local = np.full((N_CORES, T_TILES, P, K), -1.0, np.float32)
    dst_nodes = np.zeros((N_CORES, T_TILES, P, 1), np.int32)
    node_order = np.zeros((N_CORES, T_TILES, P), np.int64)
    # dlocT[c, t, j*P + e] = d_local[c, t, e, j] (edge-slot-major, for STt)
    for c in range(N_CORES):
        for tp in range(T_TILES):
            t = perm[c, tp]
            g = c * T_TILES + t
            s, e = gs[g], ge[g]
            n = e - s
            i = np.arange(n)
            dl = node_pos[c, dst_s[s:e] - c * NODES_PER_CORE]
            src32[c, tp, i % P, i // P] = src_s[s:e]
            d_local[c, tp, i % P, i // P] = dl
            dst_nodes[c, tp, :, 0] = group_nodes[c, t]
            node_order[c, tp] = group_nodes[c, t]
            if pad_from[c, t] < P:
                node_order[c, tp, pad_from[c, t]:] = -1
    dlocT = np.ascontiguousarray(
        d_local.transpose(0, 1, 3, 2)).reshape(N_CORES, T_TILES, K * P)
    # table rows are laid out partition-major (row = (n%128)*392 + n//128)
    # so phase-0 writes are contiguous per partition; bake the permutation
    # into the gather offsets.
    src32 = ((src32 % P) * (NPAD // P) + src32 // P).astype(np.int32)
    dst_nodes = ((dst_nodes % P) * (NPAD // P)
                 + dst_nodes // P).astype(np.int32)
    return W_ext, src32, d_local, dlocT, dst_nodes, node_order, k_list


def _build_module(k_list, bias_nz, b2_nz):
    K = max(k_list)
    import concourse.bass as bass
    import concourse.mybir as mybir
    import concourse.tile as tile
    from concourse import bacc
    from concourse.masks import make_identity

    f32 = mybir.dt.float32
    bf16 = mybir.dt.bfloat16
    i32 = mybir.dt.int32

    nc = bacc.Bacc("TRN2", target_bir_lowering=False, debug=False,
                   num_devices=N_CORES, dynamic_dma_scratch_size=32768)

    x_T = nc.dram_tensor("x_T", [P, NPAD], bf16, kind="ExternalInput")
    W_ext_d = nc.dram_tensor("W_ext", [P, FE], bf16, kind="ExternalInput")
    W2_d = nc.dram_tensor("W2", [P, F_OUT], bf16, kind="ExternalInput")
    s32_d = nc.dram_tensor("src32", [T_TILES, P, K], i32,
                           kind="ExternalInput")
    dstn_d = nc.dram_tensor("dst_nodes", [T_TILES, P, 1], i32,
                            kind="ExternalInput")
    dloc_d = nc.dram_tensor("d_local", [T_TILES, P, K], bf16,
                            kind="ExternalInput")
    dlocT_d = nc.dram_tensor("d_localT", [T_TILES, K * P], bf16,
                             kind="ExternalInput")
    if bias_nz:
        bias_d = nc.dram_tensor("bias_ext", [1, FE], bf16,
                                kind="ExternalInput")
    if b2_nz:
        b2_d = nc.dram_tensor("b2_row", [1, F_OUT], bf16,
                              kind="ExternalInput")
    y_d = nc.dram_tensor("y_out", [T_TILES * P, F_OUT], f32,
                         kind="ExternalOutput")
    htab = nc.dram_tensor("htab", [NPAD, EB], bf16, kind="Internal")
    ad_tab = nc.dram_tensor("ad_tab", [NPAD, HEADS], f32, kind="Internal")

    add = mybir.AluOpType.add
    mult = mybir.AluOpType.mult
    amax = mybir.AluOpType.max
    is_eq = mybir.AluOpType.is_equal
    Exp = mybir.ActivationFunctionType.Exp

    N_CHUNKS = NPAD // (CH * P)  # 28

    with tile.TileContext(nc) as tc:
        with tc.tile_pool(name="const", bufs=1) as constp:
            W_ext_sb = constp.tile([P, FE], bf16)
            nc.sync.dma_start(W_ext_sb[:], W_ext_d.ap())
            W2_sb = constp.tile([P, F_OUT], bf16)
            nc.sync.dma_start(W2_sb[:], W2_d.ap())
            iota_f = constp.tile([P, P], f32)
            nc.gpsimd.iota(iota_f[:], pattern=[[1, P]], base=0,
                           channel_multiplier=0,
                           allow_small_or_imprecise_dtypes=True)
            iota_sb = constp.tile([P, P], bf16)
            nc.vector.tensor_copy(iota_sb[:], iota_f[:])
            iotap_f = constp.tile([P, 1], f32)
            nc.gpsimd.iota(iotap_f[:], pattern=[[0, 1]], base=0,
                           channel_multiplier=1,
                           allow_small_or_imprecise_dtypes=True)
            iotap = constp.tile([P, 1], bf16)
            nc.vector.tensor_copy(iotap[:], iotap_f[:])
            ident_f = constp.tile([P, P], f32)
            make_identity(nc, ident_f[:])
            ident = constp.tile([P, P], bf16)
            nc.vector.tensor_copy(ident[:], ident_f[:])
            ones1 = constp.tile([1, P], bf16)
            nc.vector.memset(ones1[:], 1.0)
            s32_sb = constp.tile([P, T_TILES, K], i32)
            nc.sync.dma_start(s32_sb[:],
                              s32_d.ap().rearrange("t p k -> p t k"))
            dstn_sb = constp.tile([P, T_TILES], i32)
            nc.sync.dma_start(dstn_sb[:],
                              dstn_d.ap().rearrange("t p one -> p (t one)"))
            dloc_sb = constp.tile([P, T_TILES, K], bf16)
            nc.sync.dma_start(dloc_sb[:],
                              dloc_d.ap().rearrange("t p k -> p t k"))
            if bias_nz or b2_nz:
                ones_sb = constp.tile([1, P], bf16)
                nc.vector.memset(ones_sb[:], 1.0)
            if bias_nz:
                bias_sb = constp.tile([1, FE], bf16)
                nc.sync.dma_start(bias_sb[:], bias_d.ap())
            if b2_nz:
                b2_sb = constp.tile([1, F_OUT], bf16)
                nc.sync.dma_start(b2_sb[:], b2_d.ap())
            ad_acc = constp.tile([P, NPAD // P, HEADS], f32)
            y_acc = constp.tile([P, T_TILES, F_OUT], f32)

            # ---- phase 0: htab = [x@W_ext | as]; ad_acc = ad ----
            # 3 node-tiles share one PSUM bank so the PSUM->SBUF copies
            # amortize the DVE per-op overhead.
            with (
                tc.tile_pool(name="xt", bufs=3) as xtp,
                tc.tile_pool(name="hx", bufs=3) as hxp,
                tc.tile_pool(name="p0ps", bufs=4, space="PSUM") as p0ps,
            ):
                for c in range(N_CHUNKS):
                    xt = xtp.tile([P, CH * P], bf16)
                    nc.scalar.dma_start(
                        xt[:], x_T.ap()[:, c * CH * P:(c + 1) * CH * P])
                    hrow = hxp.tile([P, CH, EB], bf16, tag="hrow")
                    hrow_f32 = hrow[:].bitcast(f32)
                    for j0 in range(0, CH, 3):
                        nj = min(3, CH - j0)
                        ps = p0ps.tile([P, 3, FE], f32)
                        for j in range(j0, j0 + nj):
                            nc.tensor.matmul(
                                ps[:, j - j0, :],
                                lhsT=xt[:, j * P:(j + 1) * P],
                                rhs=W_ext_sb[:], start=True,
                                stop=not bias_nz)
                            if bias_nz:
                                nc.tensor.matmul(ps[:, j - j0, :],
                                                 lhsT=ones_sb[:],
                                                 rhs=bias_sb[:], start=False,
                                                 stop=True)
                        nc.vector.tensor_copy(
                            hrow[:, j0:j0 + nj, 0:F_IN],
                            ps[:, 0:nj, 0:F_IN])
                        nc.vector.tensor_copy(
                            hrow_f32[:, j0:j0 + nj, AS_OFF:AS_OFF + HEADS],
                            ps[:, 0:nj, F_IN:F_IN + HEADS])
                        nc.scalar.activation(
                            out=ad_acc[:, c * CH + j0:c * CH + j0 + nj, :],
                            in_=ps[:, 0:nj, F_IN + HEADS:FE],
                            func=mybir.ActivationFunctionType.Copy)
                    htab_pt = htab.ap().rearrange("(p t) e -> p t e", p=P)
                    nc.sync.dma_start(
                        htab_pt[:, c * CH:(c + 1) * CH, :], hrow[:])
                    if (c + 1) % 7 == 0:  # quarters: after chunks 6,13,20,27
                        ad_pt = ad_tab.ap().rearrange("(p t) e -> p t e", p=P)
                        nc.sync.dma_start(
                            ad_pt[:, (c - 6) * CH:(c + 1) * CH, :],
                            ad_acc[:, (c - 6) * CH:(c + 1) * CH, :])

            # ---- phase 1: per destination tile ----
            BC = 512  # bcast-matmul chunk (one PSUM bank of f32)
            with (
                tc.tile_pool(name="g", bufs=6) as gp,
                tc.tile_pool(name="msgs", bufs=3) as mp,
                tc.tile_pool(name="S", bufs=2) as sp,
                tc.tile_pool(name="STt", bufs=2) as stp,
                tc.tile_pool(name="dlT", bufs=2) as dlp,
                tc.tile_pool(name="agg", bufs=2, space="PSUM") as aggp,
                tc.tile_pool(name="bcps", bufs=2, space="PSUM") as bcpsp,
                tc.tile_pool(name="adps", bufs=2, space="PSUM") as adpsp,
                tc.tile_pool(name="small", bufs=4) as smallp,
                tc.tile_pool(name="tr", bufs=1, space="PSUM") as trp,
                tc.tile_pool(name="yps", bufs=1, space="PSUM") as ypsp,
            ):
                for t in range(T_TILES):
                    Kt = k_list[t]
                    adt = smallp.tile([P, HEADS], f32, tag="adt")
                    nc.gpsimd.indirect_dma_start(
                        out=adt[:], out_offset=None, in_=ad_tab.ap(),
                        in_offset=bass.IndirectOffsetOnAxis(
                            ap=dstn_sb[:, t:t + 1], axis=0))
                    g = gp.tile([P, Kt, EB], bf16, tag="g")
                    for j in range(Kt):
                        nc.gpsimd.indirect_dma_start(
                            out=g[:, j, :], out_offset=None,
                            in_=htab.ap(),
                            in_offset=bass.IndirectOffsetOnAxis(
                                ap=s32_sb[:, t, j:j + 1], axis=0))
                    adtb = smallp.tile([P, HEADS], bf16, tag="adtb")
                    nc.vector.tensor_copy(adtb[:], adt[:])

                    # one-hot scatter matrix S[p=e, (k, d)] in bf16
                    S = sp.tile([P, Kt, P], bf16, tag="S")
                    nc.vector.tensor_tensor(
                        out=S[:],
                        in0=iota_sb[:].unsqueeze(1).to_broadcast([P, Kt, P]),
                        in1=dloc_sb[:, t, 0:Kt].unsqueeze(2).to_broadcast(
                            [P, Kt, P]),
                        op=is_eq)
                    # STt[d, (j e)] = (d == dloc[e, j]) built directly:
                    # dlocT row broadcast across partitions via PE, then one
                    # is_equal against the partition-index iota.
                    dlT_row = dlp.tile([1, Kt * P], bf16, tag="dlr")
                    nc.sync.dma_start(dlT_row[:],
                                      dlocT_d.ap()[t:t + 1, 0:Kt * P])
                    dlT = dlp.tile([P, Kt * P], bf16, tag="dlT")
                    for q0 in range(0, Kt * P, BC):
                        qn = min(BC, Kt * P - q0)
                        bps = bcpsp.tile([P, BC], f32)
                        nc.tensor.matmul(bps[:, 0:qn], lhsT=ones1[:],
                                         rhs=dlT_row[:, q0:q0 + qn],
                                         start=True, stop=True)
                        nc.vector.tensor_copy(dlT[:, q0:q0 + qn],
                                              bps[:, 0:qn])
                    STt = stp.tile([P, Kt * P], bf16, tag="STt")
                    nc.vector.tensor_tensor(
                        out=STt[:],
                        in0=iotap[:].to_broadcast([P, Kt * P]),
                        in1=dlT[:], op=is_eq)
                    # alpha_d per edge: adps[:, j*4:(j+1)*4] = STt_j.T @ adtb
                    adps = adpsp.tile([P, Kt * HEADS], f32, tag="adps")
                    for j in range(Kt):
                        nc.tensor.matmul(
                            adps[:, j * HEADS:(j + 1) * HEADS],
                            lhsT=STt[:, j * P:(j + 1) * P], rhs=adtb[:],
                            start=True, stop=True)

                    # ex = exp(leakyrelu(alpha_s + alpha_d)) per edge
                    gf = g[:].bitcast(f32)   # [P, Kt, 68]
                    ex = smallp.tile([P, Kt, HEADS], f32, tag="ex")
                    nc.vector.tensor_tensor(
                        out=ex[:], in0=gf[:, :, AS_OFF:AS_OFF + HEADS],
                        in1=adps[:].rearrange("p (k h) -> p k h", k=Kt),
                        op=add)
                    nc.vector.scalar_tensor_tensor(
                        out=ex[:], in0=ex[:], scalar=NEG, in1=ex[:],
                        op0=mult, op1=amax)
                    exb = smallp.tile([P, Kt, HEADS], bf16, tag="exb")
                    nc.scalar.activation(out=exb[:], in_=ex[:], func=Exp)

                    msgs = mp.tile([P, Kt, FM], bf16, tag="msgs")
                    nc.vector.tensor_tensor(
                        out=msgs[:, :, 0:F_IN].rearrange(
                            "p k (h f) -> p k h f", h=HEADS),
                        in0=g[:, :, 0:F_IN].rearrange(
                            "p k (h f) -> p k h f", h=HEADS),
                        in1=exb[:].unsqueeze(3).to_broadcast(
                            [P, Kt, HEADS, HIDDEN]),
                        op=mult)
                    nc.vector.tensor_copy(msgs[:, :, F_IN:FM], exb[:])

                    ps = aggp.tile([P, FM], f32)
                    for j in range(Kt):
                        nc.tensor.matmul(ps[:], lhsT=S[:, j, :],
                                         rhs=msgs[:, j, :],
                                         start=(j == 0), stop=(j == K - 1))

                    rec = smallp.tile([P, HEADS], f32, tag="rec")
                    nc.vector.tensor_scalar_add(out=rec[:],
                                                in0=ps[:, F_IN:FM],
                                                scalar1=1e-16)
                    nc.vector.reciprocal(rec[:], rec[:])
                    zn = smallp.tile([P, F_IN], f32, tag="zn")
                    nc.vector.tensor_tensor(
                        out=zn[:].rearrange("p (h f) -> p h f", h=HEADS),
                        in0=ps[:, 0:F_IN].rearrange("p (h f) -> p h f",
                                                    h=HEADS),
                        in1=rec[:].unsqueeze(2).to_broadcast(
                            [P, HEADS, HIDDEN]),
                        op=mult)
                    # ELU(z) = max(z, exp(min(z,0)) - 1)
                    tmp = smallp.tile([P, F_IN], f32, tag="tmp")
                    nc.vector.tensor_scalar_min(out=tmp[:], in0=zn[:],
                                                scalar1=0.0)
                    nc.scalar.activation(out=tmp[:], in_=tmp[:], func=Exp)
                    znb = smallp.tile([P, F_IN], bf16, tag="znb")
                    nc.vector.scalar_tensor_tensor(
                        out=znb[:], in0=tmp[:], scalar=-1.0, in1=zn[:],
                        op0=add, op1=amax)

                    pt = trp.tile([P, P], bf16, tag="pt")
                    nc.tensor.transpose(out=pt[:], in_=znb[:],
                                        identity=ident[:])
                    znT = smallp.tile([P, P], bf16, tag="znT")
                    nc.vector.tensor_copy(znT[:], pt[:])
                    yp = ypsp.tile([P, F_OUT], f32, tag="yp")
                    nc.tensor.matmul(yp[:], lhsT=znT[:], rhs=W2_sb[:],
                                     start=True, stop=not b2_nz)
                    if b2_nz:
                        nc.tensor.matmul(yp[:], lhsT=ones_sb[:], rhs=b2_sb[:],
                                         start=False, stop=True)
                    nc.vector.tensor_copy(y_acc[:, t, :], yp[:])
            nc.sync.dma_start(
                y_d.ap().rearrange("(t p) f -> p t f", p=P), y_acc[:])

    nc.compile()
    return nc


_MODULE_CACHE = {}


def _get_module(k_list, bias_nz, b2_nz):
    key = (tuple(k_list), bias_nz, b2_nz)
    if key not in _MODULE_CACHE:
        _MODULE_CACHE[key] = _build_module(k_list, bias_nz, b2_nz)
    return _MODULE_CACHE[key]


def _ensure_ntff_hook():
    """The axon NTFF profile hook lives in antenv.axon_hooks, which this
    image's antenv package lacks; shim it so trace=True works."""
    try:
        import antenv.axon_hooks  # noqa: F401
        return
    except ImportError:
        pass
    import types

    import antenv

    mod = types.ModuleType("antenv.axon_hooks")
    holder = {"h": None}
    mod.set_axon_ntff_profile_hook = lambda h: holder.__setitem__("h", h)
    mod.get_axon_ntff_profile_hook = lambda: holder["h"]
    try:
        from trn_agent_boot.trn_boot import _ntff_profile_via_ctypes
        holder["h"] = _ntff_profile_via_ctypes("/opt/axon/libaxon_pjrt.so")
    except Exception:
        pass
    sys.modules["antenv.axon_hooks"] = mod
    antenv.axon_hooks = mod


def kernel(x, edge_index, edge_weight, W, a_src, a_dst, bias, W2, b2,
           _trace=False):
    import ml_dtypes
    from concourse.bass_utils import run_bass_kernel_spmd

    bf = ml_dtypes.bfloat16
    if _trace:
        _ensure_ntff_hook()

    x = np.asarray(x, np.float32)
    W = np.asarray(W, np.float32)
    a_src = np.asarray(a_src, np.float32)
    a_dst = np.asarray(a_dst, np.float32)
    bias = np.asarray(bias, np.float32)
    W2 = np.asarray(W2, np.float32)
    b2 = np.asarray(b2, np.float32)

    W_ext, src32, d_local, dlocT, dst_nodes, node_order, k_list = _prep(
        edge_index, W, a_src, a_dst)

    bias_nz = bool(np.any(bias))
    b2_nz = bool(np.any(b2))
    nc = _get_module(k_list, bias_nz, b2_nz)

    x_T = np.zeros((P, NPAD), bf)
    x_T[:, :N_NODES] = x.T.astype(bf)

    in_maps = []
    for c in range(N_CORES):
        m = {
            "x_T": x_T,
            "W_ext": W_ext.astype(bf),
            "W2": W2.astype(bf),
            "src32": np.ascontiguousarray(src32[c]),
            "dst_nodes": np.ascontiguousarray(dst_nodes[c]),
            "d_local": np.ascontiguousarray(d_local[c].astype(bf)),
            "d_localT": np.ascontiguousarray(dlocT[c].astype(bf)),
        }
        if bias_nz:
            be = np.zeros((1, FE), np.float32)
            be[0, :F_IN] = bias
            m["bias_ext"] = be.astype(bf)
        if b2_nz:
            m["b2_row"] = b2.reshape(1, F_OUT).astype(bf)
        in_maps.append(m)

    res = run_bass_kernel_spmd(nc, in_maps, core_ids=list(range(N_CORES)),
                               trace=_trace)
    out = np.zeros((N_NODES, F_OUT), np.float32)
    for c in range(N_CORES):
        y = res.results[c]["y_out"].reshape(T_TILES * P, F_OUT)
        idx = node_order[c].reshape(-1)
        valid = idx >= 0
        out[idx[valid]] = y[valid]
    if _trace:
        kernel.last_results = res
    return out


# revision 8
# speedup vs baseline: 1.1857x; 1.0193x over previous
"""GAT layer (nn_GAT_40037685133531) as a Trainium2 Bass kernel on 8 NeuronCores.

Strategy (graph/data parallel, no collectives):
  - Destination nodes sharded 8 ways (6250 per core), then bin-packed into
    49 balanced 128-node groups per core (serpentine deal + overflow
    concentrated into one group) so 48 of 49 groups need exactly 16
    edge slots of 128; per-position slot counts are the max over cores.
  - Phase 0 (replicated, bf16): h_ext = x @ [W | W@A_s | W@A_d] -> htab
    [NPAD, 136] bf16 rows = [h bf16 x128 | alpha_s f32 x4] (272B); alpha_d
    accumulated in SBUF and written once to ad_tab [NPAD, 4] f32.
  - Phase 1 per tile: K x [P,1]-offset indirect DMAs gather the edge rows
    by src (measured: ~1.1us/op fixed issue cost on gpsimd is the kernel
    floor; multi-offset/dma_gather/ap_gather forms all measured slower).
    alpha_d for the tile's 128 dst nodes comes from one [P,1] gather on
    ad_tab and is broadcast to edges via bf16 PE transposes of the one-hot
    S (built by DVE is_equal vs an iota). ex = exp(leakyrelu(as+ad)),
    msgs = [ex*h | ex] bf16, psum += S_j.T @ msgs_j aggregates weighted
    sums + softmax denominators. Normalize, ELU, PE-transpose, z @ W2 in
    bf16; y accumulated in SBUF and written once.
"""

import os
import sys

import numpy as np

if "/opt/trn_rl_repo" not in sys.path:
    sys.path.insert(0, "/opt/trn_rl_repo")

N_NODES = 50000
N_EDGES = 800000
F_IN = 128
HEADS = 4
HIDDEN = 32
F_OUT = 64
NEG = 0.2
N_CORES = 8
P = 128
FE = F_IN + 2 * HEADS    # 136 phase-0 psum cols: h | alpha_s | alpha_d
FM = F_IN + HEADS        # 132 message cols: ex*h | ex
EB = FE                  # bf16 elements per htab row (272 B)
AS_OFF = 64              # f32 element offset of alpha_s within a row
NODES_PER_CORE = N_NODES // N_CORES          # 6250
T_TILES = (NODES_PER_CORE + P - 1) // P      # 49
NPAD = T_TILES * 8 * P                       # 50176
CH = 14                                      # phase-0 node tiles per chunk

def _prep(edge_index, W, a_src, a_dst):
    """CPU-side: extended weights; edges sorted by (core, tile, src) and
    packed into [P, K] slot layout per destination tile."""
    src = np.asarray(edge_index[0]).astype(np.int64)
    dst = np.asarray(edge_index[1]).astype(np.int64)

    A_s = np.zeros((F_IN, HEADS), np.float32)
    A_d = np.zeros((F_IN, HEADS), np.float32)
    for h in range(HEADS):
        A_s[h * HIDDEN:(h + 1) * HIDDEN, h] = a_src[h]
        A_d[h * HIDDEN:(h + 1) * HIDDEN, h] = a_dst[h]
    W_ext = np.concatenate([W, W @ A_s, W @ A_d], axis=1).astype(np.float32)

    core_of = dst // NODES_PER_CORE
    # Bin-pack each core's dst nodes into T_TILES groups of exactly P nodes,
    # balancing edge counts (LPT): per-core edges avg NODES... ~100k/49 ~= 2041
    # < 2048, so nearly every group fits 16 slots of 128 edges.
    node_group = np.zeros((N_CORES, NODES_PER_CORE), np.int32)
    node_pos = np.zeros((N_CORES, NODES_PER_CORE), np.int32)
    group_nodes = np.zeros((N_CORES, T_TILES, P), np.int64)
    pad_from = np.zeros((N_CORES, T_TILES), np.int32)
    for c in range(N_CORES):
        lo, hi = c * NODES_PER_CORE, (c + 1) * NODES_PER_CORE
        deg = np.bincount(dst[core_of == c] - lo, minlength=NODES_PER_CORE)
        order_n = np.argsort(-deg, kind="stable")
        # serpentine deal for near-equal loads with equal node counts
        bins = [[] for _ in range(T_TILES)]
        bi, step = 0, 1
        for ln in order_n:
            while len(bins[bi]) >= P:
                bi += step
                if bi in (-1, T_TILES):
                    step = -step
                    bi += step
            bins[bi].append(ln)
            bi += step
            if bi in (-1, T_TILES):
                step = -step
                bi += step
        load = np.array([int(deg[b].sum()) for b in bins])
        # repair: push overflow (>16 slots) into the single heaviest bin by
        # swapping its light nodes for other bins' heavy nodes
        CAP = 16 * P
        b0 = int(np.argmax(load))
        for b in range(T_TILES):
            if b == b0:
                continue
            while load[b] > CAP:
                hi_i = max(range(len(bins[b])), key=lambda i: deg[bins[b][i]])
                lo_i = min(range(len(bins[b0])),
                           key=lambda i: deg[bins[b0][i]])
                nh, nl = bins[b][hi_i], bins[b0][lo_i]
                if deg[nh] <= deg[nl]:
                    break
                bins[b][hi_i], bins[b0][lo_i] = nl, nh
                load[b] += deg[nl] - deg[nh]
                load[b0] += deg[nh] - deg[nl]
        nfill = np.zeros(T_TILES, np.int32)
        for b in range(T_TILES):
            for pos, ln in enumerate(bins[b]):
                node_group[c, ln] = b
                node_pos[c, ln] = pos
                group_nodes[c, b, pos] = lo + ln
            nfill[b] = len(bins[b])
        # pad slots (tiles of the last 22 dummies) already absorbed: every
        # bin has exactly P real nodes (NODES_PER_CORE=6250 < 49*128=6272)
        for b in np.flatnonzero(nfill < P):
            group_nodes[c, b, nfill[b]:] = lo  # harmless duplicate for adt
        pad_from[c] = nfill

    group_all = (core_of * T_TILES
                 + node_group[core_of, dst - core_of * NODES_PER_CORE])
    order = np.lexsort((src, group_all))
    src_s, dst_s, group_s = src[order], dst[order], group_all[order]

    NG = N_CORES * T_TILES
    gs = np.searchsorted(group_s, np.arange(NG))
    ge = np.searchsorted(group_s, np.arange(NG), side="right")
    cnt = (ge - gs).reshape(N_CORES, T_TILES)
    # Sort each core's groups by edge count (desc): loop position t then only
    # needs k_list[t] = max over cores of the t-th largest slot count.
    perm = np.argsort(-cnt, axis=1)
    cnt_sorted = np.take_along_axis(cnt, perm, axis=1)
    k_list = [max(1, int(np.max((cnt_sorted[:, t] + P - 1) // P)))
              for t in range(T_TILES)]
    K = max(k_list)

    src32 = np.zeros((N_CORES, T_TILES, P, K), np.int32)
    d_local = np.full((N_CORES, T_TILES, P, K), -1.0, np.float32)
    dst_nodes = np.zeros((N_CORES, T_TILES, P, 1), np.int32)
    node_order = np.zeros((N_CORES, T_TILES, P), np.int64)
    # dlocT[c, t, j*P + e] = d_local[c, t, e, j] (edge-slot-major, for STt)
    for c in range(N_CORES):
        for tp in range(T_TILES):
            t = perm[c, tp]
            g = c * T_TILES + t
            s, e = gs[g], ge[g]
            n = e - s
            i = np.arange(n)
            dl = node_pos[c, dst_s[s:e] - c * NODES_PER_CORE]
            src32[c, tp, i % P, i // P] = src_s[s:e]
            d_local[c, tp, i % P, i // P] = dl
            dst_nodes[c, tp, :, 0] = group_nodes[c, t]
            node_order[c, tp] = group_nodes[c, t]
            if pad_from[c, t] < P:
                node_order[c, tp, pad_from[c, t]:] = -1
    dlocT = np.ascontiguousarray(
        d_local.transpose(0, 1, 3, 2)).reshape(N_CORES, T_TILES, K * P)
    # table rows are laid out partition-major (row = (n%128)*392 + n//128)
    # so phase-0 writes are contiguous per partition; bake the permutation
    # into the gather offsets.
    src32 = ((src32 % P) * (NPAD // P) + src32 // P).astype(np.int32)
    dst_nodes = ((dst_nodes % P) * (NPAD // P)
                 + dst_nodes // P).astype(np.int32)
    return W_ext, src32, d_local, dlocT, dst_nodes, node_order, k_list


def _build_module(k_list, bias_nz, b2_nz):
    K = max(k_list)
    import concourse.bass as bass
    import concourse.mybir as mybir
    import concourse.tile as tile
    from concourse import bacc
    from concourse.masks import make_identity

    f32 = mybir.dt.float32
    bf16 = mybir.dt.bfloat16
    i32 = mybir.dt.int32

    nc = bacc.Bacc("TRN2", target_bir_lowering=False, debug=False,
                   num_devices=N_CORES, dynamic_dma_scratch_size=32768)

    x_T = nc.dram_tensor("x_T", [P, NPAD], bf16, kind="ExternalInput")
    W_ext_d = nc.dram_tensor("W_ext", [P, FE], bf16, kind="ExternalInput")
    W2_d = nc.dram_tensor("W2", [P, F_OUT], bf16, kind="ExternalInput")
    s32_d = nc.dram_tensor("src32", [T_TILES, P, K], i32,
                           kind="ExternalInput")
    dstn_d = nc.dram_tensor("dst_nodes", [T_TILES, P, 1], i32,
                            kind="ExternalInput")
    dloc_d = nc.dram_tensor("d_local", [T_TILES, P, K], bf16,
                            kind="ExternalInput")
    dlocT_d = nc.dram_tensor("d_localT", [T_TILES, K * P], bf16,
                             kind="ExternalInput")
    if bias_nz:
        bias_d = nc.dram_tensor("bias_ext", [1, FE], bf16,
                                kind="ExternalInput")
    if b2_nz:
        b2_d = nc.dram_tensor("b2_row", [1, F_OUT], bf16,
                              kind="ExternalInput")
    y_d = nc.dram_tensor("y_out", [T_TILES * P, F_OUT], f32,
                         kind="ExternalOutput")
    htab = nc.dram_tensor("htab", [NPAD, EB], bf16, kind="Internal")
    ad_tab = nc.dram_tensor("ad_tab", [NPAD, HEADS], f32, kind="Internal")

    add = mybir.AluOpType.add
    mult = mybir.AluOpType.mult
    amax = mybir.AluOpType.max
    is_eq = mybir.AluOpType.is_equal
    Exp = mybir.ActivationFunctionType.Exp

    N_CHUNKS = NPAD // (CH * P)  # 28

    with tile.TileContext(nc) as tc:
        with tc.tile_pool(name="const", bufs=1) as constp:
            W_ext_sb = constp.tile([P, FE], bf16)
            nc.sync.dma_start(W_ext_sb[:], W_ext_d.ap())
            W2_sb = constp.tile([P, F_OUT], bf16)
            nc.sync.dma_start(W2_sb[:], W2_d.ap())
            iota_f = constp.tile([P, P], f32)
            nc.gpsimd.iota(iota_f[:], pattern=[[1, P]], base=0,
                           channel_multiplier=0,
                           allow_small_or_imprecise_dtypes=True)
            iota_sb = constp.tile([P, P], bf16)
            nc.vector.tensor_copy(iota_sb[:], iota_f[:])
            iotap_f = constp.tile([P, 1], f32)
            nc.gpsimd.iota(iotap_f[:], pattern=[[0, 1]], base=0,
                           channel_multiplier=1,
                           allow_small_or_imprecise_dtypes=True)
            iotap = constp.tile([P, 1], bf16)
            nc.vector.tensor_copy(iotap[:], iotap_f[:])
            ident_f = constp.tile([P, P], f32)
            make_identity(nc, ident_f[:])
            ident = constp.tile([P, P], bf16)
            nc.vector.tensor_copy(ident[:], ident_f[:])
            ones1 = constp.tile([1, P], bf16)
            nc.vector.memset(ones1[:], 1.0)
            s32_sb = constp.tile([P, T_TILES, K], i32)
            nc.sync.dma_start(s32_sb[:],
                              s32_d.ap().rearrange("t p k -> p t k"))
            dstn_sb = constp.tile([P, T_TILES], i32)
            nc.sync.dma_start(dstn_sb[:],
                              dstn_d.ap().rearrange("t p one -> p (t one)"))
            dloc_sb = constp.tile([P, T_TILES, K], bf16)
            nc.sync.dma_start(dloc_sb[:],
                              dloc_d.ap().rearrange("t p k -> p t k"))
            if bias_nz or b2_nz:
                ones_sb = constp.tile([1, P], bf16)
                nc.vector.memset(ones_sb[:], 1.0)
            if bias_nz:
                bias_sb = constp.tile([1, FE], bf16)
                nc.sync.dma_start(bias_sb[:], bias_d.ap())
            if b2_nz:
                b2_sb = constp.tile([1, F_OUT], bf16)
                nc.sync.dma_start(b2_sb[:], b2_d.ap())
            ad_acc = constp.tile([P, NPAD // P, HEADS], f32)
            y_acc = constp.tile([P, T_TILES, F_OUT], f32)

            # ---- phase 0: htab = [x@W_ext | as]; ad_acc = ad ----
            # 3 node-tiles share one PSUM bank so the PSUM->SBUF copies
            # amortize the DVE per-op overhead.
            with (
                tc.tile_pool(name="xt", bufs=3) as xtp,
                tc.tile_pool(name="hx", bufs=3) as hxp,
                tc.tile_pool(name="p0ps", bufs=4, space="PSUM") as p0ps,
            ):
                for c in range(N_CHUNKS):
                    xt = xtp.tile([P, CH * P], bf16)
                    nc.scalar.dma_start(
                        xt[:], x_T.ap()[:, c * CH * P:(c + 1) * CH * P])
                    hrow = hxp.tile([P, CH, EB], bf16, tag="hrow")
                    hrow_f32 = hrow[:].bitcast(f32)
                    for j0 in range(0, CH, 3):
                        nj = min(3, CH - j0)
                        ps = p0ps.tile([P, 3, FE], f32)
                        for j in range(j0, j0 + nj):
                            nc.tensor.matmul(
                                ps[:, j - j0, :],
                                lhsT=xt[:, j * P:(j + 1) * P],
                                rhs=W_ext_sb[:], start=True,
                                stop=not bias_nz)
                            if bias_nz:
                                nc.tensor.matmul(ps[:, j - j0, :],
                                                 lhsT=ones_sb[:],
                                                 rhs=bias_sb[:], start=False,
                                                 stop=True)
                        nc.vector.tensor_copy(
                            hrow[:, j0:j0 + nj, 0:F_IN],
                            ps[:, 0:nj, 0:F_IN])
                        nc.vector.tensor_copy(
                            hrow_f32[:, j0:j0 + nj, AS_OFF:AS_OFF + HEADS],
                            ps[:, 0:nj, F_IN:F_IN + HEADS])
                        nc.scalar.activation(
                            out=ad_acc[:, c * CH + j0:c * CH + j0 + nj, :],
                            in_=ps[:, 0:nj, F_IN + HEADS:FE],
                            func=mybir.ActivationFunctionType.Copy)
                    htab_pt = htab.ap().rearrange("(p t) e -> p t e", p=P)
                    nc.sync.dma_start(
                        htab_pt[:, c * CH:(c + 1) * CH, :], hrow[:])
                    if (c + 1) % 7 == 0:  # quarters: after chunks 6,13,20,27
                        ad_pt = ad_tab.ap().rearrange("(p t) e -> p t e", p=P)
                        nc.sync.dma_start(
                            ad_pt[:, (c - 6) * CH:(c + 1) * CH, :],
                            ad_acc[:, (c - 6) * CH:(c + 1) * CH, :])

            # ---- phase 1: per destination tile ----
            BC = 512  # bcast-matmul chunk (one PSUM bank of f32)
            with (
                tc.tile_pool(name="g", bufs=6) as gp,
                tc.tile_pool(name="msgs", bufs=3) as mp,
                tc.tile_pool(name="S", bufs=2) as sp,
                tc.tile_pool(name="STt", bufs=2) as stp,
                tc.tile_pool(name="dlT", bufs=2) as dlp,
                tc.tile_pool(name="agg", bufs=2, space="PSUM") as aggp,
                tc.tile_pool(name="bcps", bufs=2, space="PSUM") as bcpsp,
                tc.tile_pool(name="adps", bufs=2, space="PSUM") as adpsp,
                tc.tile_pool(name="small", bufs=4) as smallp,
                tc.tile_pool(name="tr", bufs=1, space="PSUM") as trp,
                tc.tile_pool(name="yps", bufs=1, space="PSUM") as ypsp,
            ):
                for t in range(T_TILES):
                    Kt = k_list[t]
                    adt = smallp.tile([P, HEADS], f32, tag="adt")
                    nc.gpsimd.indirect_dma_start(
                        out=adt[:], out_offset=None, in_=ad_tab.ap(),
                        in_offset=bass.IndirectOffsetOnAxis(
                            ap=dstn_sb[:, t:t + 1], axis=0))
                    g = gp.tile([P, Kt, EB], bf16, tag="g")
                    for j in range(Kt):
                        nc.gpsimd.indirect_dma_start(
                            out=g[:, j, :], out_offset=None,
                            in_=htab.ap(),
                            in_offset=bass.IndirectOffsetOnAxis(
                                ap=s32_sb[:, t, j:j + 1], axis=0))
                    adtb = smallp.tile([P, HEADS], bf16, tag="adtb")
                    nc.vector.tensor_copy(adtb[:], adt[:])

                    # one-hot scatter matrix S[p=e, (k, d)] in bf16
                    S = sp.tile([P, Kt, P], bf16, tag="S")
                    nc.vector.tensor_tensor(
                        out=S[:],
                        in0=iota_sb[:].unsqueeze(1).to_broadcast([P, Kt, P]),
                        in1=dloc_sb[:, t, 0:Kt].unsqueeze(2).to_broadcast(
                            [P, Kt, P]),
                        op=is_eq)
                    # STt[d, (j e)] = (d == dloc[e, j]) built directly:
                    # dlocT row broadcast across partitions via PE, then one
                    # is_equal against the partition-index iota.
                    dlT_row = dlp.tile([1, Kt * P], bf16, tag="dlr")
                    nc.sync.dma_start(dlT_row[:],
                                      dlocT_d.ap()[t:t + 1, 0:Kt * P])
                    # STt chunks: is_equal reads the PE partition-broadcast
                    # directly from PSUM (no intermediate copy)
                    STt = stp.tile([P, Kt * P], bf16, tag="STt")
                    for q0 in range(0, Kt * P, BC):
                        qn = min(BC, Kt * P - q0)
                        bps = bcpsp.tile([P, BC], f32)
                        nc.tensor.matmul(bps[:, 0:qn], lhsT=ones1[:],
                                         rhs=dlT_row[:, q0:q0 + qn],
                                         start=True, stop=True)
                        nc.vector.tensor_tensor(
                            out=STt[:, q0:q0 + qn],
                            in0=iotap[:].to_broadcast([P, qn]),
                            in1=bps[:, 0:qn], op=is_eq)
                    # alpha_d per edge: adps[:, j*4:(j+1)*4] = STt_j.T @ adtb
                    adps = adpsp.tile([P, Kt * HEADS], f32, tag="adps")
                    for j in range(Kt):
                        nc.tensor.matmul(
                            adps[:, j * HEADS:(j + 1) * HEADS],
                            lhsT=STt[:, j * P:(j + 1) * P], rhs=adtb[:],
                            start=True, stop=True)

                    # ex = exp(leakyrelu(alpha_s + alpha_d)) per edge
                    gf = g[:].bitcast(f32)   # [P, Kt, 68]
                    ex = smallp.tile([P, Kt, HEADS], f32, tag="ex")
                    nc.vector.tensor_tensor(
                        out=ex[:], in0=gf[:, :, AS_OFF:AS_OFF + HEADS],
                        in1=adps[:].rearrange("p (k h) -> p k h", k=Kt),
                        op=add)
                    nc.vector.scalar_tensor_tensor(
                        out=ex[:], in0=ex[:], scalar=NEG, in1=ex[:],
                        op0=mult, op1=amax)
                    exb = smallp.tile([P, Kt, HEADS], bf16, tag="exb")
                    nc.scalar.activation(out=exb[:], in_=ex[:], func=Exp)

                    msgs = mp.tile([P, Kt, FM], bf16, tag="msgs")
                    nc.vector.tensor_tensor(
                        out=msgs[:, :, 0:F_IN].rearrange(
                            "p k (h f) -> p k h f", h=HEADS),
                        in0=g[:, :, 0:F_IN].rearrange(
                            "p k (h f) -> p k h f", h=HEADS),
                        in1=exb[:].unsqueeze(3).to_broadcast(
                            [P, Kt, HEADS, HIDDEN]),
                        op=mult)
                    nc.vector.tensor_copy(msgs[:, :, F_IN:FM], exb[:])

                    ps = aggp.tile([P, FM], f32)
                    for j in range(Kt):
                        nc.tensor.matmul(ps[:], lhsT=S[:, j, :],
                                         rhs=msgs[:, j, :],
                                         start=(j == 0), stop=(j == K - 1))

                    rec = smallp.tile([P, HEADS], f32, tag="rec")
                    nc.vector.tensor_scalar_add(out=rec[:],
                                                in0=ps[:, F_IN:FM],
                                                scalar1=1e-16)
                    nc.vector.reciprocal(rec[:], rec[:])
                    zn = smallp.tile([P, F_IN], f32, tag="zn")
                    nc.vector.tensor_tensor(
                        out=zn[:].rearrange("p (h f) -> p h f", h=HEADS),
                        in0=ps[:, 0:F_IN].rearrange("p (h f) -> p h f",
                                                    h=HEADS),
                        in1=rec[:].unsqueeze(2).to_broadcast(
                            [P, HEADS, HIDDEN]),
                        op=mult)
                    # ELU(z) = max(z, exp(min(z,0)) - 1)
                    tmp = smallp.tile([P, F_IN], f32, tag="tmp")
                    nc.vector.tensor_scalar_min(out=tmp[:], in0=zn[:],
                                                scalar1=0.0)
                    nc.scalar.activation(out=tmp[:], in_=tmp[:], func=Exp)
                    znb = smallp.tile([P, F_IN], bf16, tag="znb")
                    nc.vector.scalar_tensor_tensor(
                        out=znb[:], in0=tmp[:], scalar=-1.0, in1=zn[:],
                        op0=add, op1=amax)

                    pt = trp.tile([P, P], bf16, tag="pt")
                    nc.tensor.transpose(out=pt[:], in_=znb[:],
                                        identity=ident[:])
                    znT = smallp.tile([P, P], bf16, tag="znT")
                    nc.vector.tensor_copy(znT[:], pt[:])
                    yp = ypsp.tile([P, F_OUT], f32, tag="yp")
                    nc.tensor.matmul(yp[:], lhsT=znT[:], rhs=W2_sb[:],
                                     start=True, stop=not b2_nz)
                    if b2_nz:
                        nc.tensor.matmul(yp[:], lhsT=ones_sb[:], rhs=b2_sb[:],
                                         start=False, stop=True)
                    nc.vector.tensor_copy(y_acc[:, t, :], yp[:])
            nc.sync.dma_start(
                y_d.ap().rearrange("(t p) f -> p t f", p=P), y_acc[:])

    nc.compile()
    return nc


_MODULE_CACHE = {}


def _get_module(k_list, bias_nz, b2_nz):
    key = (tuple(k_list), bias_nz, b2_nz)
    if key not in _MODULE_CACHE:
        _MODULE_CACHE[key] = _build_module(k_list, bias_nz, b2_nz)
    return _MODULE_CACHE[key]


def _ensure_ntff_hook():
    """The axon NTFF profile hook lives in antenv.axon_hooks, which this
    image's antenv package lacks; shim it so trace=True works."""
    try:
        import antenv.axon_hooks  # noqa: F401
        return
    except ImportError:
        pass
    import types

    import antenv

    mod = types.ModuleType("antenv.axon_hooks")
    holder = {"h": None}
    mod.set_axon_ntff_profile_hook = lambda h: holder.__setitem__("h", h)
    mod.get_axon_ntff_profile_hook = lambda: holder["h"]
    try:
        from trn_agent_boot.trn_boot import _ntff_profile_via_ctypes
        holder["h"] = _ntff_profile_via_ctypes("/opt/axon/libaxon_pjrt.so")
    except Exception:
        pass
    sys.modules["antenv.axon_hooks"] = mod
    antenv.axon_hooks = mod


def kernel(x, edge_index, edge_weight, W, a_src, a_dst, bias, W2, b2,
           _trace=False):
    import ml_dtypes
    from concourse.bass_utils import run_bass_kernel_spmd

    bf = ml_dtypes.bfloat16
    if _trace:
        _ensure_ntff_hook()

    x = np.asarray(x, np.float32)
    W = np.asarray(W, np.float32)
    a_src = np.asarray(a_src, np.float32)
    a_dst = np.asarray(a_dst, np.float32)
    bias = np.asarray(bias, np.float32)
    W2 = np.asarray(W2, np.float32)
    b2 = np.asarray(b2, np.float32)

    W_ext, src32, d_local, dlocT, dst_nodes, node_order, k_list = _prep(
        edge_index, W, a_src, a_dst)

    bias_nz = bool(np.any(bias))
    b2_nz = bool(np.any(b2))
    nc = _get_module(k_list, bias_nz, b2_nz)

    x_T = np.zeros((P, NPAD), bf)
    x_T[:, :N_NODES] = x.T.astype(bf)

    in_maps = []
    for c in range(N_CORES):
        m = {
            "x_T": x_T,
            "W_ext": W_ext.astype(bf),
            "W2": W2.astype(bf),
            "src32": np.ascontiguousarray(src32[c]),
            "dst_nodes": np.ascontiguousarray(dst_nodes[c]),
            "d_local": np.ascontiguousarray(d_local[c].astype(bf)),
            "d_localT": np.ascontiguousarray(dlocT[c].astype(bf)),
        }
        if bias_nz:
            be = np.zeros((1, FE), np.float32)
            be[0, :F_IN] = bias
            m["bias_ext"] = be.astype(bf)
        if b2_nz:
            m["b2_row"] = b2.reshape(1, F_OUT).astype(bf)
        in_maps.append(m)

    res = run_bass_kernel_spmd(nc, in_maps, core_ids=list(range(N_CORES)),
                               trace=_trace)
    out = np.zeros((N_NODES, F_OUT), np.float32)
    for c in range(N_CORES):
        y = res.results[c]["y_out"].reshape(T_TILES * P, F_OUT)
        idx = node_order[c].reshape(-1)
        valid = idx >= 0
        out[idx[valid]] = y[valid]
    if _trace:
        kernel.last_results = res
    return out
